# revision 39
# baseline (speedup 1.0000x reference)
"""Trainium2 Bass kernel for nn_AspectLinkModel (BERT-ish dual encoder + pairwise cosine sum).

Strategy: pure data-parallel over batch (2 batches/core x 8 cores), both encoder
calls fused into one 768-token stream per core. Feature-major activations.
Fast path (used for the standard inputs: unit LN gamma/beta, zero v/o/f2
biases): fp8e4m3 DoubleRow matmuls for the six big projections with
power-of-2 weight pre-scales compensated in existing free scale fields, and
a constant 1/sqrt(3072) cosine normalizer (unit-norm LN outputs).  Legacy
bf16 body retained as fallback for nonstandard inputs.  Output [16]
assembled on host from per-core per-sequence sums (sum-then-dot).
"""
import math
import sys
import numpy as np
import ml_dtypes

for _p in ('/opt/trn_rl_repo', '/root/.axon_site/_ro/trn_rl_repo'):
    if _p not in sys.path:
        sys.path.insert(0, _p)

import concourse.bass as bass  # noqa: E402
import concourse.tile as tile  # noqa: E402
from concourse import bacc, mybir  # noqa: E402
from concourse.bass_utils import run_bass_kernel_spmd  # noqa: E402

# Steer the ACT table-set chooser so interleaved Exp/Ln activations resolve to
# the combined natural_log_exp_and_others set instead of thrashing between
# exp_and_others and natural_log (~2.7us per reload). Set order (and therefore
# act_func_set_id indices) is preserved; we only hide exp/ln from the
# single-function sets.
import concourse.hw_specs as _hw_specs  # noqa: E402
import functools  # noqa: E402

_orig_get_tables = _hw_specs.get_activation_tables


@functools.cache
def _patched_get_tables(arch):
    src = _orig_get_tables(arch)
    out = {}
    for name, funcs in src.items():
        fs = set(funcs)
        if name == 'exp_and_others':
            fs.discard(mybir.ActivationFunctionType.Exp)
        if name == 'natural_log':
            fs.discard(mybir.ActivationFunctionType.Ln)
        out[name] = fs
    return out


_hw_specs.get_activation_tables = _patched_get_tables
if getattr(bacc, 'get_activation_tables', None) is _orig_get_tables:
    bacc.get_activation_tables = _patched_get_tables

F32 = mybir.dt.float32
BF16 = mybir.dt.bfloat16
F8 = mybir.dt.float8e4
AF = mybir.ActivationFunctionType
AX = mybir.AxisListType
DR = mybir.MatmulPerfMode.DoubleRow

L, D, H, DH, FF, EMB = 4, 768, 12, 64, 3072, 300
KC = D // 128          # 6 feature chunks
FFC = FF // 128        # 24 ffn chunks
T = 768                # tokens per core: [ctx0(256) | ctx1(256) | asp0(128) | asp1(128)]
NTS = [(0, 384), (384, 384)]
CORES = list(range(8))

QS = 1.0 / 64.0        # compensation for Wq*64, Wv*64, W1*64
KS = 0.125 / 64.0      # k path also folds the 1/8 score scale
OBIAS = -3 * math.log(2.0)   # inv = exp(-ln(cs)+OBIAS) = (1/8)/cs for Wo*8

import os
NO_DR = bool(int(os.environ.get('BASS_NO_DR', '0')))      # debug: plain fp8 MMs
# The merged two-MMs-into-one-bank + single [128,512] exp pattern faults the
# device in full-kernel context (bisected on HW); keep the per-j split.
SPLIT_EXP = bool(int(os.environ.get('BASS_SPLIT_EXP', '1')))
FAST_LAYERS = int(os.environ.get('BASS_FAST_LAYERS', str(L)))  # debug: truncate
SKIP_SECT = set(os.environ.get('BASS_SKIP', '').split(','))    # debug: {attn,ffn}


def _mm8(nc, p, lhs3, rhs3, first, last):
    """DoubleRow fp8 matmul pair (or two plain fp8 MMs under BASS_NO_DR).

    lhs3: [128, 2, m] stationary slice; rhs3: [128, 2, n] moving slice.
    """
    if NO_DR:
        for t in range(2):
            nc.tensor.matmul(p, lhs3[:, t, :], rhs3[:, t, :],
                             start=(first and t == 0), stop=(last and t == 1))
    else:
        nc.tensor.matmul(p, lhs3, rhs3, start=first, stop=last, perf_mode=DR)

_NC_CACHE = {}


def _build_nc_fast():
    nc = bacc.Bacc("TRN2", target_bir_lowering=False, debug=False)
    dd = {}
    dd['xT_d'] = nc.dram_tensor("xT", [384, T], BF16, kind="ExternalInput")
    dd['fc1_d'] = nc.dram_tensor("fc1p", [384, D], BF16, kind="ExternalInput")
    dd['posT_d'] = nc.dram_tensor("posT", [D, T], BF16, kind="ExternalInput")
    dd['bq_d'] = nc.dram_tensor("bq", [L, D], F32, kind="ExternalInput")
    dd['bk_d'] = nc.dram_tensor("bk8", [L, D], F32, kind="ExternalInput")
    dd['bf1_d'] = nc.dram_tensor("bf1", [L, FF], F32, kind="ExternalInput")
    dd['Wq8_d'] = nc.dram_tensor("Wq8", [L, 128, KC * D], F8, kind="ExternalInput")
    dd['Wk8_d'] = nc.dram_tensor("Wk8", [L, 128, KC * D], F8, kind="ExternalInput")
    dd['Wv8_d'] = nc.dram_tensor("Wv8", [L, 128, KC * D], F8, kind="ExternalInput")
    dd['Wo8_d'] = nc.dram_tensor("Wo8", [L, 128, KC * D], F8, kind="ExternalInput")
    dd['W18_d'] = nc.dram_tensor("W18", [L, FFC, 128, D], F8, kind="ExternalInput")
    dd['W28_d'] = nc.dram_tensor("W28", [L, 128, FFC * D], F8, kind="ExternalInput")
    dd['out_d'] = nc.dram_tensor("out", [128, 120], F32, kind="ExternalOutput")
    with tile.TileContext(nc) as tc:
        _body_fast(nc, tc, dd)
    nc.compile()
    return nc


def _body_fast(nc, tc, dd):
    import contextlib

    @contextlib.contextmanager
    def low_priority(offset=1500):
        # Tile's list scheduler orders ready instructions by bass_priority
        # (emission counter).  Raising it pushes deferred work out of the
        # critical boundary window; data deps still guarantee correctness.
        orig = tc.cur_priority
        tc.cur_priority = orig + offset
        try:
            yield
        finally:
            tc.cur_priority = orig

    ctx = contextlib.ExitStack()
    with ctx:
        sb = ctx.enter_context(tc.tile_pool(name="sb", bufs=1))
        ps = ctx.enter_context(tc.tile_pool(name="ps", bufs=1, space="PSUM"))

        # ---------------- constants ----------------
        ones_bf = sb.tile([128, 128], BF16, name="ones_bf", tag="const", bufs=4)
        nc.vector.memset(ones_bf[:], 1.0)
        eps12 = sb.tile([128, 1], F32, name="eps12", tag="const_e", bufs=2)
        nc.vector.memset(eps12[:], 1e-12)
        obias = sb.tile([128, 1], F32, name="obias", tag="const_o", bufs=2)
        nc.vector.memset(obias[:], OBIAS)

        def col_tile(name, dram_row, n):
            t = sb.tile([128, n], F32, name=name, tag="cols", bufs=30)
            nc.sync.dma_start(t[:], dram_row.rearrange("(c p) -> p c", p=128))
            return t

        def bank(name):
            return ps.tile([128, 512], F32, name=name, tag="bank", bufs=8)

        def stat(name):
            return sb.tile([128, 384], F32, name=name, tag="stat", bufs=6)

        def stbf(name):
            return sb.tile([128, 384], BF16, name=name, tag="stbf", bufs=4)

        # persistent activation tiles
        qT = sb.tile([128, KC * T], BF16, name="qT", tag="qT", bufs=1)
        kT = sb.tile([128, KC * T], BF16, name="kT", tag="kT", bufs=1)
        vtok = sb.tile([128, KC * T], BF16, name="vtok", tag="vtok", bufs=1)
        oT8 = sb.tile([128, KC * T], F8, name="oT8", tag="oT8", bufs=1)
        f18 = sb.tile([128, FFC * T], F8, name="f18", tag="f18", bufs=1)
        f183 = f18.rearrange("p (f t) -> p f t", f=FFC)
        oT83 = oT8.rearrange("p (c t) -> p c t", c=KC)

        def hpool(name):
            return sb.tile([128, KC * T], BF16, name=name, tag="hbf", bufs=2)

        def h8pool(name):
            return sb.tile([128, KC * T], F8, name=name, tag="h8", bufs=2)

        def w8pool(name):
            return sb.tile([128, KC * D], F8, name=name, tag="w8", bufs=7)

        def w1pool(name):
            return sb.tile([128, D], F8, name=name, tag="w1", bufs=10)

        def s_tile(name):
            return sb.tile([128, KC * T], BF16, name=name, tag="spre", bufs=1)

        # ---------------- LayerNorm (dual output, gamma=1/beta=0) ----------------
        # Phase 1: stats + rstd for BOTH halves up-front (keeps the ACT queue
        # inside the ln/exp table set and off the DVE tail's critical path).
        # Phase 2 (critical): h8 = s * rstd_bf16 -> fp8, WITHOUT the mean
        # subtraction - |mean*rstd| ~ 1% which is below the fp8 noise floor,
        # and only the GEMM inputs see it.  6 cheap muls per half, split
        # DVE/GpSimd so k-chunk pairs complete early.
        # The exact bf16 h ((s-mean)*rstd_f32, residual/outb path) is emitted
        # lazily off the critical path.
        # out_bf is the offset-carrying residual hr = s*rstd (the per-token
        # offset mean*rstd cancels in the NEXT LN's own mean subtraction).
        # Only the outb sums need it removed: corr_col!=None additionally
        # exports per-half segment sums of mean*rstd for the host to subtract.
        def layer_norm(pref, s, out_bf, out_f8, corr_col=None):
            rstds, means, rstd_bfs = [], [], []
            s3 = s.rearrange("p (c t) -> p c t", c=KC)
            o3 = out_f8.rearrange("p (c t) -> p c t", c=KC) if out_f8 is not None else None
            ob3 = out_bf.rearrange("p (c t) -> p c t", c=KC)
            # phase A: stats matmuls + means (both halves)
            halves = []
            for ih, (o, w) in enumerate(NTS):
                S1 = bank(f"{pref}_S1_{ih}")
                S2 = bank(f"{pref}_S2_{ih}")
                for c in range(KC):
                    sq = stbf(f"{pref}_sq{ih}_{c}")
                    sqeng = nc.vector if c % 2 == 0 else nc.gpsimd
                    sqeng.tensor_mul(sq[:], s[:, c * T + o:c * T + o + w],
                                     s[:, c * T + o:c * T + o + w])
                    nc.tensor.matmul(S1[:, 0:384], ones_bf[:],
                                     s[:, c * T + o:c * T + o + w],
                                     start=(c == 0), stop=(c == KC - 1))
                    nc.tensor.matmul(S2[:, 0:384], ones_bf[:], sq[:],
                                     start=(c == 0), stop=(c == KC - 1))
                halves.append((S1, S2))
            # phase B: var -> rstd for both halves (ACT stays in ln/exp set)
            for ih, (o, w) in enumerate(NTS):
                S1, S2 = halves[ih]
                m2 = stat(f"{pref}_m2{ih}")
                nc.scalar.mul(m2[:], S2[:, 0:384], 1.0 / D)
                mean = sb.tile([128, 384], BF16, name=f"{pref}_mean{ih}",
                               tag="meanbf", bufs=4)
                nc.scalar.mul(mean[:], S1[:, 0:384], 1.0 / D)
                var = stat(f"{pref}_var{ih}")
                nc.vector.tensor_mul(var[:], mean[:], mean[:])
                nc.vector.tensor_sub(var[:], m2[:], var[:])
                lnv = stat(f"{pref}_lnv{ih}")
                nc.scalar.activation(lnv[:], var[:], AF.Ln, bias=eps12[:])
                rstd_bf = sb.tile([128, 384], BF16, name=f"{pref}_rstdb{ih}",
                                  tag="rstdbf", bufs=4)
                nc.scalar.activation(rstd_bf[:], lnv[:], AF.Exp, scale=-0.5)
                rstd = sb.tile([128, 384], F32, name=f"{pref}_rstd{ih}",
                               tag="rstd", bufs=4)
                nc.scalar.activation(rstd[:], lnv[:], AF.Exp, scale=-0.5)
                rstds.append(rstd)
                means.append(mean)
                rstd_bfs.append(rstd_bf)
            # phase C: h8 = s * rstd_bf on DVE (critical for next GEMMs)
            if out_f8 is not None:
                for ih, (o, w) in enumerate(NTS):
                    nc.vector.tensor_mul(
                        o3[:, :, o:o + w], s3[:, :, o:o + w],
                        rstd_bfs[ih][:, None, :].broadcast_to([128, KC, w]))
            # residual hr = s * rstd_f32 on gpsimd, deprioritized
            with low_priority():
                for ih, (o, w) in enumerate(NTS):
                    nc.gpsimd.tensor_mul(
                        ob3[:, :, o:o + w], s3[:, :, o:o + w],
                        rstds[ih][:, None, :].broadcast_to([128, KC, w]))
                if corr_col is not None:
                    segs = [(0, 0, 256), (0, 256, 128), (1, 0, 128),
                            (1, 128, 128), (1, 256, 128)]
                    for si, (ih, so, sw) in enumerate(segs):
                        mr = sb.tile([128, 384], BF16, name=f"{pref}_mr{si}",
                                     tag="mrbf", bufs=4)
                        nc.vector.tensor_mul(mr[:, 0:sw],
                                             means[ih][:, so:so + sw],
                                             rstds[ih][:, so:so + sw])
                        nc.vector.reduce_sum(
                            outb[:, corr_col + si:corr_col + si + 1],
                            mr[:, 0:sw], axis=AX.X)

        # ---------------- embed: fc1 + pos + LN ----------------
        xT_bf = [sb.tile([128, T], BF16, name=f"xT{c}", tag="xTw", bufs=6)
                 for c in range(3)]
        fc1_bf = [sb.tile([128, D], BF16, name=f"fc1w{c}", tag="xTw", bufs=6)
                  for c in range(3)]
        for c in range(3):
            nc.sync.dma_start(xT_bf[c][:], dd['xT_d'][c * 128:(c + 1) * 128, :])
            nc.sync.dma_start(fc1_bf[c][:], dd['fc1_d'][c * 128:(c + 1) * 128, :])
        posT = sb.tile([128, KC * T], BF16, name="posT", tag="posT", bufs=1)
        for c in range(KC):
            nc.sync.dma_start(posT[:, c * T:(c + 1) * T],
                              dd['posT_d'][c * 128:(c + 1) * 128, :])

        s0 = s_tile("s_emb")
        for i, (o, w) in enumerate(NTS):
            for mc in range(KC):
                p = bank(f"emb_{mc}_{i}")
                for kc in range(3):
                    nc.tensor.matmul(p[:, 0:384],
                                     fc1_bf[kc][:, mc * 128:(mc + 1) * 128],
                                     xT_bf[kc][:, o:o + w],
                                     start=(kc == 0), stop=(kc == 2))
                nc.vector.tensor_add(s0[:, mc * T + o:mc * T + o + w], p[:, 0:384],
                                     posT[:, mc * T + o:mc * T + o + w])
        outb = sb.tile([128, 120], F32, name="outb", tag="outb", bufs=1)
        nc.vector.memset(outb[:], 0.0)

        h = hpool("h0")
        h8 = h8pool("h80")
        layer_norm("ln_emb", s0, h, h8)

        pending_outb = []

        def emit_outb(hs, l):
            with low_priority():
                for c in range(KC):
                    base = (l * KC + c) * 4
                    nc.vector.reduce_sum(
                        outb[:, base:base + 2],
                        hs[:, c * T:c * T + 512].rearrange("p (s q) -> p s q", s=2),
                        axis=AX.X)
                    nc.vector.reduce_sum(
                        outb[:, base + 2:base + 4],
                        hs[:, c * T + 512:c * T + 768].rearrange("p (s q) -> p s q",
                                                                 s=2),
                        axis=AX.X)

        # ---------------- transformer layers ----------------
        for l in range(FAST_LAYERS):
            wq8 = w8pool(f"wq8_{l}")
            wk8 = w8pool(f"wk8_{l}")
            wv8 = w8pool(f"wv8_{l}")
            wo8 = w8pool(f"wo8_{l}")
            for t_, d_ in ((wq8, dd['Wq8_d']), (wk8, dd['Wk8_d']),
                           (wv8, dd['Wv8_d']), (wo8, dd['Wo8_d'])):
                nc.sync.dma_start(t_[:], d_[l])
            w2all = sb.tile([128, FFC * D], F8, name=f"w2all{l}", tag="w2all", bufs=1)
            nc.sync.dma_start(w2all[:], dd['W28_d'][l])

            bq = col_tile(f"bq{l}", dd['bq_d'][l], KC)
            bk = col_tile(f"bk{l}", dd['bk_d'][l], KC)
            bf1c = col_tile(f"bf1{l}", dd['bf1_d'][l], FFC)

            h83 = h8.rearrange("p (c t) -> p c t", c=KC)
            wq83 = wq8.rearrange("p (c d) -> p c d", c=KC)
            wk83 = wk8.rearrange("p (c d) -> p c d", c=KC)
            wv83 = wv8.rearrange("p (c d) -> p c d", c=KC)
            wo83 = wo8.rearrange("p (c d) -> p c d", c=KC)
            w2all3 = w2all.rearrange("p (f d) -> p f d", f=FFC)

            # ---- Q^T, K^T, V — half-outer so the second half's matmuls never
            # head-block the first half's in the PE stream ----
            for i, (o, w) in enumerate(NTS):
                for wn, dst, w83, bias_col, scl in (('q', qT, wq83, bq, QS),
                                                    ('k', kT, wk83, bk, KS)):
                    for mc in range(KC):
                        p = bank(f"p{wn}{l}_{mc}_{i}")
                        for kp in range(0, KC, 2):
                            _mm8(nc, p[:, 0:384],
                                 w83[:, kp:kp + 2, mc * 128:(mc + 1) * 128],
                                 h83[:, kp:kp + 2, o:o + w],
                                 kp == 0, kp == KC - 2)
                        if i == 0:
                            nc.scalar.activation(dst[:, mc * T + o:mc * T + o + w],
                                                 p[:, 0:384], AF.Identity,
                                                 bias=bias_col[:, mc:mc + 1],
                                                 scale=scl)
                        else:
                            nc.vector.tensor_scalar(
                                dst[:, mc * T + o:mc * T + o + w], p[:, 0:384], scl,
                                bias_col[:, mc:mc + 1], mybir.AluOpType.mult,
                                mybir.AluOpType.add)
                for tch in range(KC):
                    p = bank(f"pv{l}_{tch}_{i}")
                    for kp in range(0, KC, 2):
                        _mm8(nc, p[:, 0:384],
                             h83[:, kp:kp + 2, tch * 128:(tch + 1) * 128],
                             wv83[:, kp:kp + 2, o:o + w],
                             kp == 0, kp == KC - 2)
                    nc.vector.tensor_scalar_mul(
                        vtok[:, tch * T + o:tch * T + o + w], p[:, 0:384], QS)

            # previous layer's outb sums (deprioritized; engines idle-fill)
            if pending_outb:
                hs_prev, lprev = pending_outb.pop()
                emit_outb(hs_prev, lprev)

            # ---- attention (merged [128,512] exp instructions) ----
            if 'attn' in SKIP_SECT:
                nc.vector.memset(oT8[:], 0.06)
            for c in range(KC if 'attn' not in SKIP_SECT else 0):
                for si, (qo, kts) in enumerate(((0, (0, 1)), (256, (2, 3)))):
                    av = bank(f"av{l}_{si}_{c}")
                    cs = bank(f"cs{l}_{si}_{c}")
                    eTs = []
                    for ki, kt in enumerate(kts):
                        eT = sb.tile([128, 512], BF16, name=f"eT{l}_{si}_{c}_{ki}",
                                     tag="eT", bufs=5)
                        if SPLIT_EXP:
                            for j in range(2):
                                sp = bank(f"sp{l}_{si}_{c}_{ki}_{j}")
                                nc.tensor.matmul(
                                    sp[:, 0:256],
                                    kT[j * 64:j * 64 + 64,
                                       c * T + kt * 128:c * T + (kt + 1) * 128],
                                    qT[j * 64:j * 64 + 64, c * T + qo:c * T + qo + 256],
                                    start=True, stop=True)
                                nc.scalar.activation(eT[:, j * 256:(j + 1) * 256],
                                                     sp[:, 0:256], AF.Exp)
                        else:
                            sp = bank(f"sp{l}_{si}_{c}_{ki}")
                            for j in range(2):
                                nc.tensor.matmul(
                                    sp[:, j * 256:(j + 1) * 256],
                                    kT[j * 64:j * 64 + 64,
                                       c * T + kt * 128:c * T + (kt + 1) * 128],
                                    qT[j * 64:j * 64 + 64, c * T + qo:c * T + qo + 256],
                                    start=True, stop=True)
                            nc.scalar.activation(eT[:], sp[:], AF.Exp)
                        eTs.append(eT)
                    for j in range(2):
                        hh = 2 * c + j
                        r0 = j * 64
                        for ki in range(2):
                            nc.tensor.matmul(
                                av[r0:r0 + 64, 0:256],
                                vtok[:, kts[ki] * T + hh * 64:kts[ki] * T + hh * 64 + 64],
                                eTs[ki][:, j * 256:(j + 1) * 256],
                                start=(ki == 0), stop=(ki == 1),
                                tile_position=(0, r0))
                        for ki in range(2):
                            nc.tensor.matmul(
                                cs[r0:r0 + 64, 0:256], ones_bf[:, 0:64],
                                eTs[ki][:, j * 256:(j + 1) * 256],
                                start=(ki == 0), stop=(ki == 1),
                                tile_position=(0, r0))
                    # inv = exp(-ln(8*cs)) = (1/8)/cs  (Wo pre-scaled by 8)
                    lnc = sb.tile([128, 256], F32, name=f"lnc{l}_{si}_{c}",
                                  tag="inv", bufs=4)
                    nc.scalar.activation(lnc[:], cs[:, 0:256], AF.Ln, scale=8.0)
                    inv = sb.tile([128, 256], F32, name=f"inv{l}_{si}_{c}",
                                  tag="inv", bufs=4)
                    nc.scalar.activation(inv[:], lnc[:], AF.Exp, scale=-1.0)
                    nc.vector.tensor_mul(oT8[:, c * T + qo:c * T + qo + 256],
                                         av[:, 0:256], inv[:])
                for kt in (4, 5):
                    qw0 = 512 + (kt - 4) * 128
                    col0 = (kt - 4) * 128
                    eT = sb.tile([128, 512], BF16, name=f"eTa{l}_{kt}_{c}",
                                 tag="eT", bufs=5)
                    if SPLIT_EXP:
                        for j in range(2):
                            sp = bank(f"spa{l}_{kt}_{c}_{j}")
                            nc.tensor.matmul(
                                sp[:, 0:256],
                                kT[j * 64:j * 64 + 64,
                                   c * T + kt * 128:c * T + (kt + 1) * 128],
                                qT[j * 64:j * 64 + 64, c * T + 512:c * T + 768],
                                start=True, stop=True)
                            nc.scalar.activation(eT[:, j * 256:(j + 1) * 256],
                                                 sp[:, 0:256], AF.Exp)
                    else:
                        sp = bank(f"spa{l}_{kt}_{c}")
                        for j in range(2):
                            nc.tensor.matmul(
                                sp[:, j * 256:(j + 1) * 256],
                                kT[j * 64:j * 64 + 64,
                                   c * T + kt * 128:c * T + (kt + 1) * 128],
                                qT[j * 64:j * 64 + 64, c * T + 512:c * T + 768],
                                start=True, stop=True)
                        nc.scalar.activation(eT[:], sp[:], AF.Exp)
                    av = bank(f"ava{l}_{kt}_{c}")
                    cs = bank(f"csa{l}_{kt}_{c}")
                    for j in range(2):
                        hh = 2 * c + j
                        r0 = j * 64
                        nc.tensor.matmul(
                            av[r0:r0 + 64, 0:128],
                            vtok[:, kt * T + hh * 64:kt * T + hh * 64 + 64],
                            eT[:, j * 256 + col0:j * 256 + col0 + 128],
                            start=True, stop=True, tile_position=(0, r0))
                        nc.tensor.matmul(
                            cs[r0:r0 + 64, 0:128], ones_bf[:, 0:64],
                            eT[:, j * 256 + col0:j * 256 + col0 + 128],
                            start=True, stop=True, tile_position=(0, r0))
                    lnc = sb.tile([128, 256], F32, name=f"lnca{l}_{kt}_{c}",
                                  tag="inv", bufs=4)
                    nc.scalar.activation(lnc[:, 0:128], cs[:, 0:128], AF.Ln,
                                         scale=8.0)
                    inv = sb.tile([128, 256], F32, name=f"inva{l}_{kt}_{c}",
                                  tag="inv", bufs=4)
                    nc.scalar.activation(inv[:, 0:128], lnc[:, 0:128], AF.Exp,
                                         scale=-1.0)
                    nc.vector.tensor_mul(oT8[:, c * T + qw0:c * T + qw0 + 128],
                                         av[:, 0:128], inv[:, 0:128])

            # ---- O projection + residual -> s1, LN1 ----
            s1 = s_tile(f"s1_{l}")
            for i, (o, w) in enumerate(NTS):
                for mc in range(KC):
                    p = bank(f"po{l}_{mc}_{i}")
                    for kp in range(0, KC, 2):
                        _mm8(nc, p[:, 0:384],
                             wo83[:, kp:kp + 2, mc * 128:(mc + 1) * 128],
                             oT83[:, kp:kp + 2, o:o + w],
                             kp == 0, kp == KC - 2)
                    nc.vector.tensor_add(s1[:, mc * T + o:mc * T + o + w],
                                         p[:, 0:384],
                                         h[:, mc * T + o:mc * T + o + w])
            hmid = hpool(f"hmid{l}")
            hmid8 = h8pool(f"hmid8_{l}")
            layer_norm(f"ln1_{l}", s1, hmid, hmid8)
            hmid83 = hmid8.rearrange("p (c t) -> p c t", c=KC)

            # ---- FFN (half-outer; W1 tiles re-fetched per half) ----
            if 'ffn' in SKIP_SECT:
                nc.vector.memset(f18[:], 0.06)
            for i, (o, w) in enumerate(NTS):
                if 'ffn' in SKIP_SECT:
                    break
                for fc in range(FFC):
                    w1c = w1pool(f"w1_{l}_{i}_{fc}")
                    nc.sync.dma_start(w1c[:], dd['W18_d'][l, fc])
                    w1c3 = w1c.rearrange("p (c d) -> p c d", c=KC)
                    p = bank(f"pf1{l}_{fc}_{i}")
                    for kp in range(0, KC, 2):
                        _mm8(nc, p[:, 0:384],
                             w1c3[:, kp:kp + 2, :],
                             hmid83[:, kp:kp + 2, o:o + w],
                             kp == 0, kp == KC - 2)
                    nc.scalar.activation(f18[:, fc * T + o:fc * T + o + w],
                                         p[:, 0:384], AF.Gelu_apprx_tanh,
                                         bias=bf1c[:, fc:fc + 1], scale=QS)
            s2 = s_tile(f"s2_{l}")
            for i, (o, w) in enumerate(NTS):
                for mc in range(KC):
                    p = bank(f"pf2{l}_{mc}_{i}")
                    for fp in range(0, FFC, 2):
                        _mm8(nc, p[:, 0:384],
                             w2all3[:, fp:fp + 2, mc * 128:(mc + 1) * 128],
                             f183[:, fp:fp + 2, o:o + w],
                             fp == 0, fp == FFC - 2)
                    nc.vector.tensor_add(s2[:, mc * T + o:mc * T + o + w],
                                         p[:, 0:384],
                                         hmid[:, mc * T + o:mc * T + o + w])
            hout = hpool(f"hL{l}")
            hout8 = h8pool(f"hL8_{l}") if l < L - 1 else None
            layer_norm(f"ln2_{l}", s2, hout, hout8, corr_col=96 + 5 * l)
            pending_outb.append((hout, l))
            h = hout
            h8 = hout8

        while pending_outb:
            hs_prev, lprev = pending_outb.pop()
            emit_outb(hs_prev, lprev)
        nc.sync.dma_start(dd['out_d'][:], outb[:])


def _build_nc(n_layers=L, taps=(), with_bias=False, unit_norm=False, unit_gb=False):
    nc = bacc.Bacc("TRN2", target_bir_lowering=False, debug=False)

    dd = {}
    dd['xT_d'] = nc.dram_tensor("xT", [384, T], BF16, kind="ExternalInput")
    dd['fc1_d'] = nc.dram_tensor("fc1p", [384, D], BF16, kind="ExternalInput")
    dd['posT_d'] = nc.dram_tensor("posT", [D, T], BF16, kind="ExternalInput")
    dd['embg_d'] = nc.dram_tensor("emb_g", [D], F32, kind="ExternalInput")
    dd['embb_d'] = nc.dram_tensor("emb_b", [D], F32, kind="ExternalInput")
    dd['Wq_d'] = nc.dram_tensor("Wq", [L, D, D], BF16, kind="ExternalInput")
    dd['Wk_d'] = nc.dram_tensor("Wk", [L, D, D], BF16, kind="ExternalInput")
    dd['Wv_d'] = nc.dram_tensor("Wv", [L, D, D], BF16, kind="ExternalInput")
    dd['Wo_d'] = nc.dram_tensor("Wo", [L, D, D], BF16, kind="ExternalInput")
    dd['bq_d'] = nc.dram_tensor("bq", [L, D], F32, kind="ExternalInput")
    dd['bk_d'] = nc.dram_tensor("bk8", [L, D], F32, kind="ExternalInput")
    dd['bv_d'] = nc.dram_tensor("bv", [L, D], F32, kind="ExternalInput")
    dd['bo_d'] = nc.dram_tensor("bo", [L, D], F32, kind="ExternalInput")
    dd['l1g_d'] = nc.dram_tensor("ln1_g", [L, D], F32, kind="ExternalInput")
    dd['l1b_d'] = nc.dram_tensor("ln1_b", [L, D], F32, kind="ExternalInput")
    dd['l2g_d'] = nc.dram_tensor("ln2_g", [L, D], F32, kind="ExternalInput")
    dd['l2b_d'] = nc.dram_tensor("ln2_b", [L, D], F32, kind="ExternalInput")
    dd['W1r_d'] = nc.dram_tensor("W1r", [L, FFC, 128, D], BF16, kind="ExternalInput")
    dd['W2_d'] = nc.dram_tensor("W2", [L, FF, D], BF16, kind="ExternalInput")
    dd['bf1_d'] = nc.dram_tensor("bf1", [L, FF], F32, kind="ExternalInput")
    dd['bf2_d'] = nc.dram_tensor("bf2", [L, D], F32, kind="ExternalInput")
    dd['out_d'] = nc.dram_tensor("out", [128, 96], F32, kind="ExternalOutput")
    tap_d = {}
    for tname in taps:
        tap_d[tname] = nc.dram_tensor("tap_" + tname, [128, 6 * T], BF16,
                                      kind="ExternalOutput")

    with tile.TileContext(nc) as tc:
        _body(nc, tc, dd, n_layers, taps, tap_d, with_bias, unit_norm, unit_gb)
    nc.compile()
    return nc


def _body(nc, tc, dd, n_layers, taps, tap_d, with_bias, unit_norm, unit_gb):
    import contextlib
    ctx = contextlib.ExitStack()
    with ctx:
        sb = ctx.enter_context(tc.tile_pool(name="sb", bufs=1))
        ps = ctx.enter_context(tc.tile_pool(name="ps", bufs=1, space="PSUM"))

        # ---------------- constants ----------------
        ones_bf = sb.tile([128, 128], BF16, name="ones_bf", tag="const", bufs=4)
        nc.vector.memset(ones_bf[:], 1.0)
        eps12 = sb.tile([128, 1], F32, name="eps12", tag="const_e", bufs=2)
        nc.vector.memset(eps12[:], 1e-12)
        eps16 = sb.tile([128, 1], F32, name="eps16", tag="const_e2", bufs=2)
        nc.vector.memset(eps16[:], 1e-16)

        def col_tile(name, dram_row, n):
            t = sb.tile([128, n], F32, name=name, tag="cols", bufs=44)
            nc.sync.dma_start(t[:], dram_row.rearrange("(c p) -> p c", p=128))
            return t

        # 6 persistent big bf16 tiles [128, 6*768]; each layer uses 4 of them
        # (qT, kT, vtok, oT -> then f1 parts), rotated by 4 per layer so that
        # a layer's early writes never wait on the previous layer's F2 reads.
        bigs6 = [sb.tile([128, KC * T], BF16, name=f"bigp{i}", tag=f"pers{i}", bufs=1)
                 for i in range(6)]

        def layer_bigs(l):
            return [bigs6[(4 * l + i) % 6] for i in range(4)]

        def hpool(name):
            return sb.tile([128, KC * T], BF16, name=name, tag="hmid", bufs=2)

        def saved_tile(name):
            return sb.tile([128, KC * T], BF16, name=name, tag="saved", bufs=4)

        def s_tile(name):
            return sb.tile([128, KC * T], BF16, name=name, tag="spre", bufs=1)

        def wstr(name):
            return sb.tile([128, D], BF16, name=name, tag="wstr", bufs=10)

        def bank(name, w=384):
            return ps.tile([128, w], F32, name=name, tag="bank", bufs=8)

        def stat(name):
            return sb.tile([128, 384], F32, name=name, tag="stat", bufs=6)

        def statw(name):
            return sb.tile([128, D], F32, name=name, tag="statw", bufs=3)

        def stbf(name):
            return sb.tile([128, 384], BF16, name=name, tag="stbf", bufs=4)


        # ---------------- LayerNorm (token-half pipelined) ----------------
        def layer_norm(pref, s, g_col, gc, b_col, bc, out):
            """s: [128,4608] bf16 pre-LN sums; out: [128,4608] bf16.
            Processed independently per token half so consumers of half 0
            can start while half 1 is still in flight."""
            for ih, (o, w) in enumerate(NTS):
                S1 = bank(f"{pref}_S1_{ih}")
                S2 = bank(f"{pref}_S2_{ih}")
                for c in range(KC):
                    sq = stbf(f"{pref}_sq{ih}_{c}")
                    nc.gpsimd.tensor_mul(sq[:], s[:, c * T + o:c * T + o + w],
                                         s[:, c * T + o:c * T + o + w])
                    nc.tensor.matmul(S1[:], ones_bf[:], s[:, c * T + o:c * T + o + w],
                                     start=(c == 0), stop=(c == KC - 1))
                    nc.tensor.matmul(S2[:], ones_bf[:], sq[:],
                                     start=(c == 0), stop=(c == KC - 1))
                m2 = stat(f"{pref}_m2{ih}")
                nc.scalar.mul(m2[:], S2[:], 1.0 / D)
                var = stat(f"{pref}_var{ih}")
                mean = stat(f"{pref}_mean{ih}")
                nc.scalar.mul(mean[:], S1[:], 1.0 / D)
                nc.vector.tensor_mul(var[:], mean[:], mean[:])
                nc.vector.tensor_sub(var[:], m2[:], var[:])
                # rstd = exp(-0.5 * ln(var + eps))  (ACT, keeps DVE free)
                lnv = stat(f"{pref}_lnv{ih}")
                nc.scalar.activation(lnv[:], var[:], AF.Ln, bias=eps12[:])
                rstd = stat(f"{pref}_rstd{ih}")
                nc.scalar.activation(rstd[:], lnv[:], AF.Exp, scale=-0.5)
                for c in range(KC):
                    eng = nc.gpsimd if c >= 4 else nc.vector
                    t1 = stat(f"{pref}_t1_{ih}_{c}")
                    eng.tensor_sub(t1[:], s[:, c * T + o:c * T + o + w], mean[:])
                    if unit_gb:
                        # gamma==1/beta==0: the multiply writes the bf16 output
                        # directly, dropping the ACT affine from the chain
                        eng.tensor_mul(out[:, c * T + o:c * T + o + w], t1[:], rstd[:])
                    else:
                        eng.tensor_mul(t1[:], t1[:], rstd[:])
                        nc.scalar.activation(out[:, c * T + o:c * T + o + w], t1[:],
                                             AF.Identity, bias=b_col[:, bc:bc + 1],
                                             scale=g_col[:, gc:gc + 1])

        # ---------------- embed: fc1 + pos + LN ----------------
        xT_bf = [wstr(f"xT{c}") for c in range(3)]
        fc1_bf = [wstr(f"fc1w{c}") for c in range(3)]
        for c in range(3):
            nc.sync.dma_start(xT_bf[c][:], dd['xT_d'][c * 128:(c + 1) * 128, :])
            nc.sync.dma_start(fc1_bf[c][:], dd['fc1_d'][c * 128:(c + 1) * 128, :])
        posT = bigs6[0]
        for c in range(KC):
            nc.sync.dma_start(posT[:, c * T:(c + 1) * T],
                              dd['posT_d'][c * 128:(c + 1) * 128, :])
        embg = col_tile("embg", dd['embg_d'][:], KC)
        embb = col_tile("embb", dd['embb_d'][:], KC)

        s0 = s_tile("s_emb")
        for i, (o, w) in enumerate(NTS):
            for mc in range(KC):
                p = bank(f"emb_{mc}_{i}")
                for kc in range(3):
                    nc.tensor.matmul(p[:], fc1_bf[kc][:, mc * 128:(mc + 1) * 128],
                                     xT_bf[kc][:, o:o + w],
                                     start=(kc == 0), stop=(kc == 2))
                nc.vector.tensor_add(s0[:, mc * T + o:mc * T + o + w], p[:],
                                     posT[:, mc * T + o:mc * T + o + w])
        h = hpool("h0")
        layer_norm("ln_emb", s0, embg, 0, embb, 0, h)
        if 'h0' in taps:
            nc.sync.dma_start(tap_d['h0'][:], h[:])

        saved = []
        N2 = None if unit_norm else [bank(f"n2_{i}") for i in range(2)]
        outb = sb.tile([128, 96], F32, name="outb", tag="outb", bufs=1)
        nc.vector.memset(outb[:], 0.0)

        # ---------------- transformer layers ----------------
        for l in range(n_layers):
            bq = col_tile(f"bq{l}", dd['bq_d'][l], KC)
            bk = col_tile(f"bk{l}", dd['bk_d'][l], KC)
            l1g = col_tile(f"l1g{l}", dd['l1g_d'][l], KC)
            l1b = col_tile(f"l1b{l}", dd['l1b_d'][l], KC)
            l2g = col_tile(f"l2g{l}", dd['l2g_d'][l], KC)
            l2b = col_tile(f"l2b{l}", dd['l2b_d'][l], KC)
            bf1c = col_tile(f"bf1{l}", dd['bf1_d'][l], FFC)
            if with_bias:
                bvc = col_tile(f"bvc{l}", dd['bv_d'][l], KC)
                boc = col_tile(f"boc{l}", dd['bo_d'][l], KC)
                bf2c = col_tile(f"bf2c{l}", dd['bf2_d'][l], KC)

            # ---- Q^T, K^T (feature-major, K pre-scaled 1/8) ----
            lb = layer_bigs(l)
            qT = lb[0]
            kT = lb[1]
            for dst, W_d, bias_col, scl, wn in ((qT, dd['Wq_d'], bq, 1.0, 'q'),
                                                (kT, dd['Wk_d'], bk, 0.125, 'k')):
                w_bf = [wstr(f"w{wn}{l}_{c}") for c in range(KC)]
                for c in range(KC):
                    nc.sync.dma_start(w_bf[c][:], W_d[l, c * 128:(c + 1) * 128, :])
                for i, (o, w) in enumerate(NTS):
                    for mc in range(KC):
                        p = bank(f"p{wn}{l}_{mc}_{i}")
                        for kc in range(KC):
                            nc.tensor.matmul(p[:], w_bf[kc][:, mc * 128:(mc + 1) * 128],
                                             h[:, kc * T + o:kc * T + o + w],
                                             start=(kc == 0), stop=(kc == KC - 1))
                        if i == 0:
                            nc.scalar.activation(dst[:, mc * T + o:mc * T + o + w], p[:],
                                                 AF.Identity, bias=bias_col[:, mc:mc + 1],
                                                 scale=scl)
                        else:
                            nc.vector.tensor_scalar(
                                dst[:, mc * T + o:mc * T + o + w], p[:], scl,
                                bias_col[:, mc:mc + 1], mybir.AluOpType.mult,
                                mybir.AluOpType.add)

            # ---- V (token-major: [token_chunk rows, feature cols]) ----
            vtok = lb[2]
            wv_bf = [wstr(f"wv{l}_{c}") for c in range(KC)]
            for c in range(KC):
                nc.sync.dma_start(wv_bf[c][:], dd['Wv_d'][l, c * 128:(c + 1) * 128, :])
            for tch in range(KC):
                for i, (o, w) in enumerate(NTS):
                    p = bank(f"pv{l}_{tch}_{i}")
                    for kc in range(KC):
                        nc.tensor.matmul(p[:], h[:, kc * T + tch * 128:kc * T + (tch + 1) * 128],
                                         wv_bf[kc][:, o:o + w],
                                         start=(kc == 0), stop=(kc == KC - 1))
                    nc.vector.tensor_copy(vtok[:, tch * T + o:tch * T + o + w], p[:])

            # ---- attention (heads paired per feature chunk) ----
            # chunk-outer ordering: oT chunks complete in order c=0,1,2... so
            # the O-projection's kc-ascending accumulation overlaps attention.
            oT = lb[3]
            for c in range(KC):
                for si, (qo, kts) in enumerate(((0, (0, 1)), (256, (2, 3)))):
                    av = bank(f"av{l}_{si}_{c}", 256)
                    cs = bank(f"cs{l}_{si}_{c}", 256)
                    eTs = []
                    for ki, kt in enumerate(kts):
                        eT = sb.tile([128, 512], BF16, name=f"eT{l}_{si}_{c}_{ki}",
                                     tag="eT", bufs=5)
                        for j in range(2):
                            sp = bank(f"sp{l}_{si}_{c}_{ki}_{j}", 256)
                            nc.tensor.matmul(
                                sp[:],
                                kT[j * 64:j * 64 + 64, c * T + kt * 128:c * T + (kt + 1) * 128],
                                qT[j * 64:j * 64 + 64, c * T + qo:c * T + qo + 256],
                                start=True, stop=True)
                            nc.scalar.activation(eT[:, j * 256:(j + 1) * 256], sp[:], AF.Exp)
                        eTs.append(eT)
                    for j in range(2):
                        hh = 2 * c + j
                        r0 = j * 64
                        for ki in range(2):
                            nc.tensor.matmul(av[r0:r0 + 64, :],
                                             vtok[:, kts[ki] * T + hh * 64:kts[ki] * T + hh * 64 + 64],
                                             eTs[ki][:, j * 256:(j + 1) * 256],
                                             start=(ki == 0), stop=(ki == 1),
                                             tile_position=(0, r0))
                        for ki in range(2):
                            nc.tensor.matmul(cs[r0:r0 + 64, :], ones_bf[:, 0:64],
                                             eTs[ki][:, j * 256:(j + 1) * 256],
                                             start=(ki == 0), stop=(ki == 1),
                                             tile_position=(0, r0))
                    # inv = exp(-ln(cs)) on ACT (frees DVE); multiply on DVE (PSUM)
                    lnc = sb.tile([128, 256], F32, name=f"lnc{l}_{si}_{c}",
                                  tag="inv", bufs=4)
                    nc.scalar.activation(lnc[:], cs[:], AF.Ln)
                    inv = sb.tile([128, 256], F32, name=f"inv{l}_{si}_{c}",
                                  tag="inv", bufs=4)
                    nc.scalar.activation(inv[:], lnc[:], AF.Exp, scale=-1.0)
                    nc.vector.tensor_mul(oT[:, c * T + qo:c * T + qo + 256], av[:], inv[:])
                for kt in (4, 5):
                    qw0 = 512 + (kt - 4) * 128
                    col0 = (kt - 4) * 128
                    eT = sb.tile([128, 512], BF16, name=f"eTa{l}_{kt}_{c}",
                                 tag="eT", bufs=5)
                    for j in range(2):
                        sp = bank(f"spa{l}_{kt}_{c}_{j}", 256)
                        nc.tensor.matmul(
                            sp[:],
                            kT[j * 64:j * 64 + 64, c * T + kt * 128:c * T + (kt + 1) * 128],
                            qT[j * 64:j * 64 + 64, c * T + 512:c * T + 768],
                            start=True, stop=True)
                        nc.scalar.activation(eT[:, j * 256:(j + 1) * 256], sp[:], AF.Exp)
                    av = bank(f"ava{l}_{kt}_{c}", 128)
                    cs = bank(f"csa{l}_{kt}_{c}", 128)
                    for j in range(2):
                        hh = 2 * c + j
                        r0 = j * 64
                        nc.tensor.matmul(av[r0:r0 + 64, :],
                                         vtok[:, kt * T + hh * 64:kt * T + hh * 64 + 64],
                                         eT[:, j * 256 + col0:j * 256 + col0 + 128],
                                         start=True, stop=True, tile_position=(0, r0))
                        nc.tensor.matmul(cs[r0:r0 + 64, :], ones_bf[:, 0:64],
                                         eT[:, j * 256 + col0:j * 256 + col0 + 128],
                                         start=True, stop=True, tile_position=(0, r0))
                    lnc = sb.tile([128, 256], F32, name=f"lnca{l}_{kt}_{c}",
                                  tag="inv", bufs=4)
                    nc.scalar.activation(lnc[:, 0:128], cs[:], AF.Ln)
                    inv = sb.tile([128, 256], F32, name=f"inva{l}_{kt}_{c}",
                                  tag="inv", bufs=4)
                    nc.scalar.activation(inv[:, 0:128], lnc[:, 0:128], AF.Exp, scale=-1.0)
                    nc.vector.tensor_mul(oT[:, c * T + qw0:c * T + qw0 + 128],
                                         av[:], inv[:, 0:128])

            if with_bias:
                for c in range(KC):
                    nc.scalar.activation(oT[:, c * T:(c + 1) * T],
                                         oT[:, c * T:(c + 1) * T], AF.Identity,
                                         bias=bvc[:, c:c + 1])

            # ---- O projection + residual -> s1, LN1 -> hmid ----
            wo_bf = [wstr(f"wo{l}_{c}") for c in range(KC)]
            for c in range(KC):
                nc.sync.dma_start(wo_bf[c][:], dd['Wo_d'][l, c * 128:(c + 1) * 128, :])
            s1 = s_tile(f"s1_{l}")
            for i, (o, w) in enumerate(NTS):
                for mc in range(KC):
                    p = bank(f"po{l}_{mc}_{i}")
                    for kc in range(KC):
                        nc.tensor.matmul(p[:], wo_bf[kc][:, mc * 128:(mc + 1) * 128],
                                         oT[:, kc * T + o:kc * T + o + w],
                                         start=(kc == 0), stop=(kc == KC - 1))
                    nc.vector.tensor_add(s1[:, mc * T + o:mc * T + o + w], p[:],
                                         h[:, mc * T + o:mc * T + o + w])
                    if with_bias:
                        nc.scalar.activation(s1[:, mc * T + o:mc * T + o + w],
                                             s1[:, mc * T + o:mc * T + o + w],
                                             AF.Identity, bias=boc[:, mc:mc + 1])
            hmid = hpool(f"hmid{l}")
            layer_norm(f"ln1_{l}", s1, l1g, 0, l1b, 0, hmid)

            # ---- FFN ----
            w2all = sb.tile([128, FFC * T], BF16, name=f"w2all{l}", tag="w2all", bufs=1)
            for fc in range(FFC):
                nc.sync.dma_start(w2all[:, fc * T:(fc + 1) * T],
                                  dd['W2_d'][l, fc * 128:(fc + 1) * 128, :])
            f1 = lb
            for fc in range(FFC):
                w1c = wstr(f"w1_{l}_{fc}")
                nc.sync.dma_start(w1c[:], dd['W1r_d'][l, fc])
                ftile, foff = f1[fc // 6], (fc % 6) * T
                for i, (o, w) in enumerate(NTS):
                    p = bank(f"pf1{l}_{fc}_{i}")
                    for kc in range(KC):
                        nc.tensor.matmul(p[:], w1c[:, kc * 128:(kc + 1) * 128],
                                         hmid[:, kc * T + o:kc * T + o + w],
                                         start=(kc == 0), stop=(kc == KC - 1))
                    nc.scalar.activation(ftile[:, foff + o:foff + o + w], p[:],
                                         AF.Gelu_apprx_tanh, bias=bf1c[:, fc:fc + 1])
            s2 = s_tile(f"s2_{l}")
            for i, (o, w) in enumerate(NTS):
                for mc in range(KC):
                    p = bank(f"pf2{l}_{mc}_{i}")
                    for fc in range(FFC):
                        ftile, foff = f1[fc // 6], (fc % 6) * T
                        nc.tensor.matmul(
                            p[:], w2all[:, fc * T + mc * 128:fc * T + (mc + 1) * 128],
                            ftile[:, foff + o:foff + o + w],
                            start=(fc == 0), stop=(fc == FFC - 1))
                    nc.vector.tensor_add(s2[:, mc * T + o:mc * T + o + w], p[:],
                                         hmid[:, mc * T + o:mc * T + o + w])
                    if with_bias:
                        nc.scalar.activation(s2[:, mc * T + o:mc * T + o + w],
                                             s2[:, mc * T + o:mc * T + o + w],
                                             AF.Identity, bias=bf2c[:, mc:mc + 1])
            hout = saved_tile(f"hL{l}")
            layer_norm(f"ln2_{l}", s2, l2g, 0, l2b, 0, hout)
            if unit_norm:
                for c in range(KC):
                    base = (l * KC + c) * 4
                    nc.vector.reduce_sum(
                        outb[:, base:base + 2],
                        hout[:, c * T:c * T + 512].rearrange("p (s q) -> p s q", s=2),
                        axis=AX.X)
                    nc.vector.reduce_sum(
                        outb[:, base + 2:base + 4],
                        hout[:, c * T + 512:c * T + 768].rearrange("p (s q) -> p s q", s=2),
                        axis=AX.X)
            if not unit_norm:
                # accumulate squared-norm contribution for the final cosine stage
                for c in range(KC):
                    for i, (o, w) in enumerate(NTS):
                        sq = stbf(f"fin_sq{l}_{c}_{i}")
                        nc.gpsimd.tensor_mul(sq[:], hout[:, c * T + o:c * T + o + w],
                                             hout[:, c * T + o:c * T + o + w])
                        nc.tensor.matmul(N2[i][:], ones_bf[:], sq[:],
                                         start=(l == 0 and c == 0),
                                         stop=(l == n_layers - 1 and c == KC - 1))
            saved.append(hout)
            h = hout
            if f"h{l + 1}" in taps:
                nc.sync.dma_start(tap_d[f"h{l + 1}"][:], hout[:])

        # ---------------- final: weighted per-seq sums ----------------
        if unit_norm:
            pass  # raw sums emitted inside the layer loop
        else:
            invn = statw("invn")
            for i, (o, w) in enumerate(NTS):
                sd = stat(f"fin_sd{i}")
                nc.scalar.activation(sd[:], N2[i][:], AF.Sqrt, bias=eps16[:])
                nc.vector.reciprocal(invn[:, o:o + w], sd[:])
            for li, hs in enumerate(saved):
                for c in range(KC):
                    wt = statw(f"fin_w{li}_{c}")
                    eng = nc.vector if (c % 2 == 0) else nc.gpsimd
                    eng.tensor_mul(wt[:], hs[:, c * T:(c + 1) * T], invn[:])
                    base = (li * KC + c) * 4
                    nc.vector.reduce_sum(outb[:, base:base + 2],
                                         wt[:, 0:512].rearrange("p (s q) -> p s q", s=2),
                                         axis=AX.X)
                    nc.vector.reduce_sum(outb[:, base + 2:base + 4],
                                         wt[:, 512:768].rearrange("p (s q) -> p s q", s=2),
                                         axis=AX.X)
        nc.sync.dma_start(dd['out_d'][:], outb[:])


def _prep_in_maps_fast(inputs):
    f = lambda k: np.ascontiguousarray(np.asarray(inputs[k], np.float32))
    bf = lambda a: np.ascontiguousarray(np.asarray(a, ml_dtypes.bfloat16))

    def q8(a, scale):
        a = np.asarray(a, np.float32) * scale
        a = np.clip(a, -240.0, 240.0)
        return np.ascontiguousarray(a.astype(ml_dtypes.float8_e4m3))

    ctx_e, asp_e = f('context_inputs_embeds'), f('aspect_inputs_embeds')
    fc1_w, fc1_b = f('fc1_w'), f('fc1_b')
    pos = f('pos_emb')

    fc1p = np.zeros((384, D), np.float32)
    fc1p[:EMB] = fc1_w
    posT = np.concatenate([pos[:256].T, pos[:256].T, pos[:128].T, pos[:128].T],
                          axis=1) + fc1_b[:, None]

    def wqkvo(key, scale):
        W = f(key)  # [L, D, D]
        W8 = W.reshape(L, KC, 128, D).transpose(0, 2, 1, 3).reshape(L, 128, KC * D)
        return q8(W8, scale)

    W1 = f('W1')
    W1r = W1.reshape(L, KC, 128, FFC, 128).transpose(0, 3, 2, 1, 4).reshape(L, FFC, 128, D)
    W2 = f('W2')
    W28 = W2.reshape(L, FFC, 128, D).transpose(0, 2, 1, 3).reshape(L, 128, FFC * D)

    common = {
        'fc1p': bf(fc1p), 'posT': bf(posT),
        'bq': f('bq'), 'bk8': f('bk') * 0.125, 'bf1': f('bf1'),
        'Wq8': wqkvo('Wq', 64.0), 'Wk8': wqkvo('Wk', 64.0),
        'Wv8': wqkvo('Wv', 64.0), 'Wo8': wqkvo('Wo', 8.0),
        'W18': q8(W1r, 64.0), 'W28': q8(W28, 1.0),
    }
    in_maps = []
    for i in range(8):
        xT = np.zeros((384, T), np.float32)
        xT[:EMB, 0:256] = ctx_e[2 * i].T
        xT[:EMB, 256:512] = ctx_e[2 * i + 1].T
        xT[:EMB, 512:640] = asp_e[2 * i].T
        xT[:EMB, 640:768] = asp_e[2 * i + 1].T
        in_maps.append({**common, 'xT': bf(xT)})
    return in_maps


def _prep_in_maps(inputs):
    f = lambda k: np.ascontiguousarray(np.asarray(inputs[k], np.float32))
    bf = lambda a: np.ascontiguousarray(np.asarray(a, ml_dtypes.bfloat16))
    ctx_e, asp_e = f('context_inputs_embeds'), f('aspect_inputs_embeds')
    fc1_w, fc1_b = f('fc1_w'), f('fc1_b')
    pos = f('pos_emb')

    fc1p = np.zeros((384, D), np.float32)
    fc1p[:EMB] = fc1_w
    posT = np.concatenate([pos[:256].T, pos[:256].T, pos[:128].T, pos[:128].T],
                          axis=1) + fc1_b[:, None]
    W1 = f('W1')
    # [L, FFC, 128, 768]: W1r[l, fc, p, kc*128+c2] = W1[l, kc*128+p, fc*128+c2]
    W1r = W1.reshape(L, KC, 128, FFC, 128).transpose(0, 3, 2, 1, 4).reshape(L, FFC, 128, D)
    common = {
        'fc1p': bf(fc1p), 'posT': bf(posT),
        'emb_g': f('emb_g'), 'emb_b': f('emb_b'),
        'Wq': bf(f('Wq')), 'Wk': bf(f('Wk')), 'Wv': bf(f('Wv')), 'Wo': bf(f('Wo')),
        'bq': f('bq'), 'bk8': f('bk') * 0.125, 'bv': f('bv'), 'bo': f('bo'),
        'ln1_g': f('ln1_g'), 'ln1_b': f('ln1_b'),
        'ln2_g': f('ln2_g'), 'ln2_b': f('ln2_b'),
        'W1r': bf(W1r), 'W2': bf(f('W2')), 'bf1': f('bf1'), 'bf2': f('bf2'),
    }
    in_maps = []
    for i in range(8):
        xT = np.zeros((384, T), np.float32)
        xT[:EMB, 0:256] = ctx_e[2 * i].T
        xT[:EMB, 256:512] = ctx_e[2 * i + 1].T
        xT[:EMB, 512:640] = asp_e[2 * i].T
        xT[:EMB, 640:768] = asp_e[2 * i + 1].T
        in_maps.append({**common, 'xT': bf(xT)})
    return in_maps


def _postprocess(results, unit_norm=False):
    scale = 1.0 / 3072.0 if unit_norm else 1.0
    out = np.zeros(16, np.float32)
    for i, r in enumerate(results):
        A = r['out'].reshape(128, 24, 4)
        for j in range(2):
            out[2 * i + j] = scale * float(np.sum(
                A[:, :, 2 + j].astype(np.float64) * A[:, :, j].astype(np.float64)))
    return out


def get_nc(n_layers=L, taps=(), with_bias=False, unit_norm=False, unit_gb=False,
           fast=False):
    key = (n_layers, tuple(taps), with_bias, unit_norm, unit_gb, fast)
    if key not in _NC_CACHE:
        if fast:
            _NC_CACHE[key] = _build_nc_fast()
        else:
            _NC_CACHE[key] = _build_nc(n_layers, taps, with_bias, unit_norm, unit_gb)
    return _NC_CACHE[key]


def _build_flags(inputs):
    wb = any(float(np.abs(np.asarray(inputs[k])).max()) > 0
             for k in ('bv', 'bo', 'bf2'))
    ugb = all(np.all(np.asarray(inputs[g]) == 1.0) and
              np.all(np.asarray(inputs[b]) == 0.0)
              for g, b in (('emb_g', 'emb_b'), ('ln1_g', 'ln1_b'),
                           ('ln2_g', 'ln2_b')))
    fast = ugb and not wb
    if fast:
        return dict(fast=True)
    return dict(with_bias=wb, unit_norm=False, unit_gb=ugb)


def _postprocess_fast(results):
    # outb[:, :96] holds per-seq sums of hr = s*rstd (offset-carrying);
    # outb[0, 96:116] holds per-layer segment sums of mean*rstd, which the
    # host subtracts to recover sums of the true unit-norm LN outputs.
    # Cosine normalizer is the constant 1/3072.
    out = np.zeros(16, np.float32)
    for i, r in enumerate(results):
        raw = r['out']
        A = raw[:, :96].reshape(128, 4, 6, 4).astype(np.float64)
        c5 = raw[0, 96:116].reshape(4, 5).astype(np.float64)
        corr = np.zeros((4, 4))
        corr[:, 0] = c5[:, 0]            # ctx0 = h0[0:256]
        corr[:, 1] = c5[:, 1] + c5[:, 2]  # ctx1 = h0[256:384] + h1[0:128]
        corr[:, 2] = c5[:, 3]            # asp0 = h1[128:256]
        corr[:, 3] = c5[:, 4]            # asp1 = h1[256:384]
        A = A - corr[None, :, None, :]
        for j in range(2):
            out[2 * i + j] = (1.0 / 3072.0) * float(
                np.sum(A[:, :, :, 2 + j] * A[:, :, :, j]))
    return out


def kernel(**inputs):
    flags = _build_flags(inputs)
    fast = flags.get('fast', False)
    nc = get_nc(**flags)
    in_maps = _prep_in_maps_fast(inputs) if fast else _prep_in_maps(inputs)
    last_err = None
    for attempt in range(3):
        try:
            res = run_bass_kernel_spmd(nc, in_maps, CORES)
            if fast:
                return _postprocess_fast(res.results)
            return _postprocess(res.results, False)
        except Exception as e:  # transient NRT_EXEC_UNIT_UNRECOVERABLE flakes
            last_err = e
            import time
            time.sleep(15)
    raise last_err


if __name__ == "__main__":
    d = np.load('/root/problem/inputs_cache.npz')
    out = kernel(**{k: d[k] for k in d.files})
    ref = np.load('/root/problem/ref_out.npy')
    rel = np.abs(out - ref) / np.abs(ref)
    print("out:", out)
    print("rel err:", rel.max())



# revision 40
# speedup vs baseline: 1.1961x; 1.1961x over previous
"""Trainium2 Bass kernel for nn_AspectLinkModel (BERT-ish dual encoder + pairwise cosine sum).

Strategy: pure data-parallel over batch (2 batches/core x 8 cores), both encoder
calls fused into one 768-token stream per core. Feature-major activations.
Fast path (used for the standard inputs: unit LN gamma/beta, zero v/o/f2
biases): fp8e4m3 DoubleRow matmuls for the six big projections with
power-of-2 weight pre-scales compensated in existing free scale fields, and
a constant 1/sqrt(3072) cosine normalizer (unit-norm LN outputs).  Legacy
bf16 body retained as fallback for nonstandard inputs.  Output [16]
assembled on host from per-core per-sequence sums (sum-then-dot).
"""
import math
import sys
import numpy as np
import ml_dtypes

for _p in ('/opt/trn_rl_repo', '/root/.axon_site/_ro/trn_rl_repo'):
    if _p not in sys.path:
        sys.path.insert(0, _p)

import concourse.bass as bass  # noqa: E402
import concourse.tile as tile  # noqa: E402
from concourse import bacc, mybir  # noqa: E402
from concourse.bass_utils import run_bass_kernel_spmd  # noqa: E402

# Steer the ACT table-set chooser so interleaved Exp/Ln activations resolve to
# the combined natural_log_exp_and_others set instead of thrashing between
# exp_and_others and natural_log (~2.7us per reload). Set order (and therefore
# act_func_set_id indices) is preserved; we only hide exp/ln from the
# single-function sets.
import concourse.hw_specs as _hw_specs  # noqa: E402
import functools  # noqa: E402

_orig_get_tables = _hw_specs.get_activation_tables


@functools.cache
def _patched_get_tables(arch):
    src = _orig_get_tables(arch)
    out = {}
    for name, funcs in src.items():
        fs = set(funcs)
        if name == 'exp_and_others':
            fs.discard(mybir.ActivationFunctionType.Exp)
        if name == 'natural_log':
            fs.discard(mybir.ActivationFunctionType.Ln)
        out[name] = fs
    return out


_hw_specs.get_activation_tables = _patched_get_tables
if getattr(bacc, 'get_activation_tables', None) is _orig_get_tables:
    bacc.get_activation_tables = _patched_get_tables

F32 = mybir.dt.float32
BF16 = mybir.dt.bfloat16
F8 = mybir.dt.float8e4
AF = mybir.ActivationFunctionType
AX = mybir.AxisListType
DR = mybir.MatmulPerfMode.DoubleRow

L, D, H, DH, FF, EMB = 4, 768, 12, 64, 3072, 300
KC = D // 128          # 6 feature chunks
FFC = FF // 128        # 24 ffn chunks
T = 768                # tokens per core: [ctx0(256) | ctx1(256) | asp0(128) | asp1(128)]
NTS = [(0, 384), (384, 384)]
CORES = list(range(8))

QS = 1.0 / 64.0        # compensation for Wq*64, Wv*64, W1*64
KS = 0.125 / 64.0      # k path also folds the 1/8 score scale
OBIAS = -3 * math.log(2.0)   # inv = exp(-ln(cs)+OBIAS) = (1/8)/cs for Wo*8

import os
NO_DR = bool(int(os.environ.get('BASS_NO_DR', '0')))      # debug: plain fp8 MMs
# The merged two-MMs-into-one-bank + single [128,512] exp pattern faults the
# device in full-kernel context (bisected on HW); keep the per-j split.
SPLIT_EXP = bool(int(os.environ.get('BASS_SPLIT_EXP', '1')))
FAST_LAYERS = int(os.environ.get('BASS_FAST_LAYERS', str(L)))  # debug: truncate
SKIP_SECT = set(os.environ.get('BASS_SKIP', '').split(','))    # debug: {attn,ffn}


def _mm8(nc, p, lhs3, rhs3, first, last):
    """DoubleRow fp8 matmul pair (or two plain fp8 MMs under BASS_NO_DR).

    lhs3: [128, 2, m] stationary slice; rhs3: [128, 2, n] moving slice.
    """
    if NO_DR:
        for t in range(2):
            nc.tensor.matmul(p, lhs3[:, t, :], rhs3[:, t, :],
                             start=(first and t == 0), stop=(last and t == 1))
    else:
        nc.tensor.matmul(p, lhs3, rhs3, start=first, stop=last, perf_mode=DR)

_NC_CACHE = {}


def _build_nc_fast():
    nc = bacc.Bacc("TRN2", target_bir_lowering=False, debug=False)
    dd = {}
    dd['xT_d'] = nc.dram_tensor("xT", [384, T], BF16, kind="ExternalInput")
    dd['fc1_d'] = nc.dram_tensor("fc1p", [384, D], BF16, kind="ExternalInput")
    dd['posT_d'] = nc.dram_tensor("posT", [D, T], BF16, kind="ExternalInput")
    dd['bq_d'] = nc.dram_tensor("bq", [L, D], F32, kind="ExternalInput")
    dd['bk_d'] = nc.dram_tensor("bk8", [L, D], F32, kind="ExternalInput")
    dd['bf1_d'] = nc.dram_tensor("bf1", [L, FF], F32, kind="ExternalInput")
    dd['Wq8_d'] = nc.dram_tensor("Wq8", [L, 128, KC * D], F8, kind="ExternalInput")
    dd['Wk8_d'] = nc.dram_tensor("Wk8", [L, 128, KC * D], F8, kind="ExternalInput")
    dd['Wv8_d'] = nc.dram_tensor("Wv8", [L, 128, KC * D], F8, kind="ExternalInput")
    dd['Wo8_d'] = nc.dram_tensor("Wo8", [L, 128, KC * D], F8, kind="ExternalInput")
    dd['W18_d'] = nc.dram_tensor("W18", [L, FFC, 128, D], F8, kind="ExternalInput")
    dd['W28_d'] = nc.dram_tensor("W28", [L, 128, FFC * D], F8, kind="ExternalInput")
    dd['out_d'] = nc.dram_tensor("out", [128, 120], F32, kind="ExternalOutput")
    with tile.TileContext(nc) as tc:
        _body_fast(nc, tc, dd)
    nc.compile()
    return nc


def _body_fast(nc, tc, dd):
    import contextlib

    @contextlib.contextmanager
    def low_priority(offset=1500):
        # Tile's list scheduler orders ready instructions by bass_priority
        # (emission counter).  Raising it pushes deferred work out of the
        # critical boundary window; data deps still guarantee correctness.
        orig = tc.cur_priority
        tc.cur_priority = orig + offset
        try:
            yield
        finally:
            tc.cur_priority = orig

    ctx = contextlib.ExitStack()
    with ctx:
        sb = ctx.enter_context(tc.tile_pool(name="sb", bufs=1))
        ps = ctx.enter_context(tc.tile_pool(name="ps", bufs=1, space="PSUM"))

        # ---------------- constants ----------------
        ones_bf = sb.tile([128, 128], BF16, name="ones_bf", tag="const", bufs=4)
        nc.vector.memset(ones_bf[:], 1.0)
        eps12 = sb.tile([128, 1], F32, name="eps12", tag="const_e", bufs=2)
        nc.vector.memset(eps12[:], 1e-12)
        obias = sb.tile([128, 1], F32, name="obias", tag="const_o", bufs=2)
        nc.vector.memset(obias[:], OBIAS)

        def col_tile(name, dram_row, n):
            t = sb.tile([128, n], F32, name=name, tag="cols", bufs=30)
            nc.sync.dma_start(t[:], dram_row.rearrange("(c p) -> p c", p=128))
            return t

        def bank(name):
            return ps.tile([128, 512], F32, name=name, tag="bank", bufs=8)

        def stat(name):
            return sb.tile([128, 384], F32, name=name, tag="stat", bufs=6)

        def stbf(name):
            return sb.tile([128, 384], BF16, name=name, tag="stbf", bufs=4)

        # persistent activation tiles
        qT = sb.tile([128, KC * T], BF16, name="qT", tag="qT", bufs=1)
        kT = sb.tile([128, KC * T], BF16, name="kT", tag="kT", bufs=1)
        vtok = sb.tile([128, KC * T], BF16, name="vtok", tag="vtok", bufs=1)
        oT8 = sb.tile([128, KC * T], F8, name="oT8", tag="oT8", bufs=1)
        f18 = sb.tile([128, FFC * T], F8, name="f18", tag="f18", bufs=1)
        f183 = f18.rearrange("p (f t) -> p f t", f=FFC)
        oT83 = oT8.rearrange("p (c t) -> p c t", c=KC)

        def hpool(name):
            return sb.tile([128, KC * T], BF16, name=name, tag="hbf", bufs=2)

        def h8pool(name):
            return sb.tile([128, KC * T], F8, name=name, tag="h8", bufs=2)

        def w8pool(name):
            return sb.tile([128, KC * D], F8, name=name, tag="w8", bufs=7)

        def w1pool(name):
            return sb.tile([128, D], F8, name=name, tag="w1", bufs=10)

        def s_tile(name):
            return sb.tile([128, KC * T], BF16, name=name, tag="spre", bufs=1)

        # ---------------- LayerNorm (dual output, gamma=1/beta=0) ----------------
        # Phase 1: stats + rstd for BOTH halves up-front (keeps the ACT queue
        # inside the ln/exp table set and off the DVE tail's critical path).
        # Phase 2 (critical): h8 = s * rstd_bf16 -> fp8, WITHOUT the mean
        # subtraction - |mean*rstd| ~ 1% which is below the fp8 noise floor,
        # and only the GEMM inputs see it.  6 cheap muls per half, split
        # DVE/GpSimd so k-chunk pairs complete early.
        # The exact bf16 h ((s-mean)*rstd_f32, residual/outb path) is emitted
        # lazily off the critical path.
        # out_bf is the offset-carrying residual hr = s*rstd (the per-token
        # offset mean*rstd cancels in the NEXT LN's own mean subtraction).
        # Only the outb sums need it removed: corr_col!=None additionally
        # exports per-half segment sums of mean*rstd for the host to subtract.
        def layer_norm(pref, s, out_bf, out_f8, corr_col=None):
            rstds, means, rstd_bfs = [], [], []
            s3 = s.rearrange("p (c t) -> p c t", c=KC)
            o3 = out_f8.rearrange("p (c t) -> p c t", c=KC) if out_f8 is not None else None
            ob3 = out_bf.rearrange("p (c t) -> p c t", c=KC)
            # phase A: stats matmuls + means (both halves)
            halves = []
            for ih, (o, w) in enumerate(NTS):
                S1 = bank(f"{pref}_S1_{ih}")
                S2 = bank(f"{pref}_S2_{ih}")
                for c in range(KC):
                    sq = stbf(f"{pref}_sq{ih}_{c}")
                    sqeng = nc.vector if c % 2 == 0 else nc.gpsimd
                    sqeng.tensor_mul(sq[:], s[:, c * T + o:c * T + o + w],
                                     s[:, c * T + o:c * T + o + w])
                    nc.tensor.matmul(S1[:, 0:384], ones_bf[:],
                                     s[:, c * T + o:c * T + o + w],
                                     start=(c == 0), stop=(c == KC - 1))
                    nc.tensor.matmul(S2[:, 0:384], ones_bf[:], sq[:],
                                     start=(c == 0), stop=(c == KC - 1))
                halves.append((S1, S2))
            # phase B: var -> rstd for both halves (ACT stays in ln/exp set)
            for ih, (o, w) in enumerate(NTS):
                S1, S2 = halves[ih]
                m2 = stat(f"{pref}_m2{ih}")
                nc.scalar.mul(m2[:], S2[:, 0:384], 1.0 / D)
                mean = sb.tile([128, 384], BF16, name=f"{pref}_mean{ih}",
                               tag="meanbf", bufs=4)
                nc.scalar.mul(mean[:], S1[:, 0:384], 1.0 / D)
                var = stat(f"{pref}_var{ih}")
                nc.vector.tensor_mul(var[:], mean[:], mean[:])
                nc.vector.tensor_sub(var[:], m2[:], var[:])
                lnv = stat(f"{pref}_lnv{ih}")
                nc.scalar.activation(lnv[:], var[:], AF.Ln, bias=eps12[:])
                rstd_bf = sb.tile([128, 384], BF16, name=f"{pref}_rstdb{ih}",
                                  tag="rstdbf", bufs=4)
                nc.scalar.activation(rstd_bf[:], lnv[:], AF.Exp, scale=-0.5)
                rstd = sb.tile([128, 384], F32, name=f"{pref}_rstd{ih}",
                               tag="rstd", bufs=4)
                nc.scalar.activation(rstd[:], lnv[:], AF.Exp, scale=-0.5)
                rstds.append(rstd)
                means.append(mean)
                rstd_bfs.append(rstd_bf)
            # phase C: h8 = s * rstd_bf on DVE (critical for next GEMMs)
            if out_f8 is not None:
                for ih, (o, w) in enumerate(NTS):
                    nc.vector.tensor_mul(
                        o3[:, :, o:o + w], s3[:, :, o:o + w],
                        rstd_bfs[ih][:, None, :].broadcast_to([128, KC, w]))
            # residual hr = s * rstd_f32, deprioritized (DVE after the h8
            # muls drain - avoids DVE/GpSimd SBUF contention entirely)
            with low_priority():
                for ih, (o, w) in enumerate(NTS):
                    nc.vector.tensor_mul(
                        ob3[:, :, o:o + w], s3[:, :, o:o + w],
                        rstds[ih][:, None, :].broadcast_to([128, KC, w]))
                if corr_col is not None:
                    segs = [(0, 0, 256), (0, 256, 128), (1, 0, 128),
                            (1, 128, 128), (1, 256, 128)]
                    for si, (ih, so, sw) in enumerate(segs):
                        mr = sb.tile([128, 384], BF16, name=f"{pref}_mr{si}",
                                     tag="mrbf", bufs=4)
                        nc.vector.tensor_mul(mr[:, 0:sw],
                                             means[ih][:, so:so + sw],
                                             rstds[ih][:, so:so + sw])
                        nc.vector.reduce_sum(
                            outb[:, corr_col + si:corr_col + si + 1],
                            mr[:, 0:sw], axis=AX.X)

        # ---------------- embed: fc1 + pos + LN ----------------
        xT_bf = [sb.tile([128, T], BF16, name=f"xT{c}", tag="xTw", bufs=6)
                 for c in range(3)]
        fc1_bf = [sb.tile([128, D], BF16, name=f"fc1w{c}", tag="xTw", bufs=6)
                  for c in range(3)]
        for c in range(3):
            nc.sync.dma_start(xT_bf[c][:], dd['xT_d'][c * 128:(c + 1) * 128, :])
            nc.sync.dma_start(fc1_bf[c][:], dd['fc1_d'][c * 128:(c + 1) * 128, :])
        posT = sb.tile([128, KC * T], BF16, name="posT", tag="posT", bufs=1)
        for c in range(KC):
            nc.sync.dma_start(posT[:, c * T:(c + 1) * T],
                              dd['posT_d'][c * 128:(c + 1) * 128, :])

        s0 = s_tile("s_emb")
        for i, (o, w) in enumerate(NTS):
            for mc in range(KC):
                p = bank(f"emb_{mc}_{i}")
                for kc in range(3):
                    nc.tensor.matmul(p[:, 0:384],
                                     fc1_bf[kc][:, mc * 128:(mc + 1) * 128],
                                     xT_bf[kc][:, o:o + w],
                                     start=(kc == 0), stop=(kc == 2))
                nc.vector.tensor_add(s0[:, mc * T + o:mc * T + o + w], p[:, 0:384],
                                     posT[:, mc * T + o:mc * T + o + w])
        outb = sb.tile([128, 120], F32, name="outb", tag="outb", bufs=1)
        nc.vector.memset(outb[:], 0.0)

        h = hpool("h0")
        h8 = h8pool("h80")
        layer_norm("ln_emb", s0, h, h8)

        pending_outb = []

        def emit_outb(hs, l):
            with low_priority():
                for c in range(KC):
                    base = (l * KC + c) * 4
                    nc.vector.reduce_sum(
                        outb[:, base:base + 2],
                        hs[:, c * T:c * T + 512].rearrange("p (s q) -> p s q", s=2),
                        axis=AX.X)
                    nc.vector.reduce_sum(
                        outb[:, base + 2:base + 4],
                        hs[:, c * T + 512:c * T + 768].rearrange("p (s q) -> p s q",
                                                                 s=2),
                        axis=AX.X)

        # ---------------- transformer layers ----------------
        for l in range(FAST_LAYERS):
            wq8 = w8pool(f"wq8_{l}")
            wk8 = w8pool(f"wk8_{l}")
            wv8 = w8pool(f"wv8_{l}")
            wo8 = w8pool(f"wo8_{l}")
            for t_, d_ in ((wq8, dd['Wq8_d']), (wk8, dd['Wk8_d']),
                           (wv8, dd['Wv8_d']), (wo8, dd['Wo8_d'])):
                nc.sync.dma_start(t_[:], d_[l])
            w2all = sb.tile([128, FFC * D], F8, name=f"w2all{l}", tag="w2all", bufs=1)
            nc.sync.dma_start(w2all[:], dd['W28_d'][l])

            bq = col_tile(f"bq{l}", dd['bq_d'][l], KC)
            bk = col_tile(f"bk{l}", dd['bk_d'][l], KC)
            bf1c = col_tile(f"bf1{l}", dd['bf1_d'][l], FFC)

            h83 = h8.rearrange("p (c t) -> p c t", c=KC)
            wq83 = wq8.rearrange("p (c d) -> p c d", c=KC)
            wk83 = wk8.rearrange("p (c d) -> p c d", c=KC)
            wv83 = wv8.rearrange("p (c d) -> p c d", c=KC)
            wo83 = wo8.rearrange("p (c d) -> p c d", c=KC)
            w2all3 = w2all.rearrange("p (f d) -> p f d", f=FFC)

            # ---- Q^T, K^T, V — half-outer so the second half's matmuls never
            # head-block the first half's in the PE stream ----
            for i, (o, w) in enumerate(NTS):
                for wn, dst, w83, bias_col, scl in (('q', qT, wq83, bq, QS),
                                                    ('k', kT, wk83, bk, KS)):
                    for mc in range(KC):
                        p = bank(f"p{wn}{l}_{mc}_{i}")
                        for kp in range(0, KC, 2):
                            _mm8(nc, p[:, 0:384],
                                 w83[:, kp:kp + 2, mc * 128:(mc + 1) * 128],
                                 h83[:, kp:kp + 2, o:o + w],
                                 kp == 0, kp == KC - 2)
                        if i == 0:
                            nc.scalar.activation(dst[:, mc * T + o:mc * T + o + w],
                                                 p[:, 0:384], AF.Identity,
                                                 bias=bias_col[:, mc:mc + 1],
                                                 scale=scl)
                        else:
                            nc.vector.tensor_scalar(
                                dst[:, mc * T + o:mc * T + o + w], p[:, 0:384], scl,
                                bias_col[:, mc:mc + 1], mybir.AluOpType.mult,
                                mybir.AluOpType.add)
                for tch in range(KC):
                    p = bank(f"pv{l}_{tch}_{i}")
                    for kp in range(0, KC, 2):
                        _mm8(nc, p[:, 0:384],
                             h83[:, kp:kp + 2, tch * 128:(tch + 1) * 128],
                             wv83[:, kp:kp + 2, o:o + w],
                             kp == 0, kp == KC - 2)
                    nc.vector.tensor_scalar_mul(
                        vtok[:, tch * T + o:tch * T + o + w], p[:, 0:384], QS)

            # previous layer's outb sums (deprioritized; engines idle-fill)
            if pending_outb:
                hs_prev, lprev = pending_outb.pop()
                emit_outb(hs_prev, lprev)

            # ---- attention (merged [128,512] exp instructions) ----
            if 'attn' in SKIP_SECT:
                nc.vector.memset(oT8[:], 0.06)
            for c in range(KC if 'attn' not in SKIP_SECT else 0):
                for si, (qo, kts) in enumerate(((0, (0, 1)), (256, (2, 3)))):
                    av = bank(f"av{l}_{si}_{c}")
                    cs = bank(f"cs{l}_{si}_{c}")
                    eTs = []
                    for ki, kt in enumerate(kts):
                        eT = sb.tile([128, 512], BF16, name=f"eT{l}_{si}_{c}_{ki}",
                                     tag="eT", bufs=5)
                        if SPLIT_EXP:
                            for j in range(2):
                                sp = bank(f"sp{l}_{si}_{c}_{ki}_{j}")
                                nc.tensor.matmul(
                                    sp[:, 0:256],
                                    kT[j * 64:j * 64 + 64,
                                       c * T + kt * 128:c * T + (kt + 1) * 128],
                                    qT[j * 64:j * 64 + 64, c * T + qo:c * T + qo + 256],
                                    start=True, stop=True)
                                nc.scalar.activation(eT[:, j * 256:(j + 1) * 256],
                                                     sp[:, 0:256], AF.Exp)
                        else:
                            sp = bank(f"sp{l}_{si}_{c}_{ki}")
                            for j in range(2):
                                nc.tensor.matmul(
                                    sp[:, j * 256:(j + 1) * 256],
                                    kT[j * 64:j * 64 + 64,
                                       c * T + kt * 128:c * T + (kt + 1) * 128],
                                    qT[j * 64:j * 64 + 64, c * T + qo:c * T + qo + 256],
                                    start=True, stop=True)
                            nc.scalar.activation(eT[:], sp[:], AF.Exp)
                        eTs.append(eT)
                    for j in range(2):
                        hh = 2 * c + j
                        r0 = j * 64
                        for ki in range(2):
                            nc.tensor.matmul(
                                av[r0:r0 + 64, 0:256],
                                vtok[:, kts[ki] * T + hh * 64:kts[ki] * T + hh * 64 + 64],
                                eTs[ki][:, j * 256:(j + 1) * 256],
                                start=(ki == 0), stop=(ki == 1),
                                tile_position=(0, r0))
                        for ki in range(2):
                            nc.tensor.matmul(
                                cs[r0:r0 + 64, 0:256], ones_bf[:, 0:64],
                                eTs[ki][:, j * 256:(j + 1) * 256],
                                start=(ki == 0), stop=(ki == 1),
                                tile_position=(0, r0))
                    # inv = exp(-ln(8*cs)) = (1/8)/cs  (Wo pre-scaled by 8)
                    lnc = sb.tile([128, 256], F32, name=f"lnc{l}_{si}_{c}",
                                  tag="inv", bufs=4)
                    nc.scalar.activation(lnc[:], cs[:, 0:256], AF.Ln, scale=8.0)
                    inv = sb.tile([128, 256], F32, name=f"inv{l}_{si}_{c}",
                                  tag="inv", bufs=4)
                    nc.scalar.activation(inv[:], lnc[:], AF.Exp, scale=-1.0)
                    nc.vector.tensor_mul(oT8[:, c * T + qo:c * T + qo + 256],
                                         av[:, 0:256], inv[:])
                for kt in (4, 5):
                    qw0 = 512 + (kt - 4) * 128
                    col0 = (kt - 4) * 128
                    eT = sb.tile([128, 512], BF16, name=f"eTa{l}_{kt}_{c}",
                                 tag="eT", bufs=5)
                    if SPLIT_EXP:
                        for j in range(2):
                            sp = bank(f"spa{l}_{kt}_{c}_{j}")
                            nc.tensor.matmul(
                                sp[:, 0:256],
                                kT[j * 64:j * 64 + 64,
                                   c * T + kt * 128:c * T + (kt + 1) * 128],
                                qT[j * 64:j * 64 + 64, c * T + 512:c * T + 768],
                                start=True, stop=True)
                            nc.scalar.activation(eT[:, j * 256:(j + 1) * 256],
                                                 sp[:, 0:256], AF.Exp)
                    else:
                        sp = bank(f"spa{l}_{kt}_{c}")
                        for j in range(2):
                            nc.tensor.matmul(
                                sp[:, j * 256:(j + 1) * 256],
                                kT[j * 64:j * 64 + 64,
                                   c * T + kt * 128:c * T + (kt + 1) * 128],
                                qT[j * 64:j * 64 + 64, c * T + 512:c * T + 768],
                                start=True, stop=True)
                        nc.scalar.activation(eT[:], sp[:], AF.Exp)
                    av = bank(f"ava{l}_{kt}_{c}")
                    cs = bank(f"csa{l}_{kt}_{c}")
                    for j in range(2):
                        hh = 2 * c + j
                        r0 = j * 64
                        nc.tensor.matmul(
                            av[r0:r0 + 64, 0:128],
                            vtok[:, kt * T + hh * 64:kt * T + hh * 64 + 64],
                            eT[:, j * 256 + col0:j * 256 + col0 + 128],
                            start=True, stop=True, tile_position=(0, r0))
                        nc.tensor.matmul(
                            cs[r0:r0 + 64, 0:128], ones_bf[:, 0:64],
                            eT[:, j * 256 + col0:j * 256 + col0 + 128],
                            start=True, stop=True, tile_position=(0, r0))
                    lnc = sb.tile([128, 256], F32, name=f"lnca{l}_{kt}_{c}",
                                  tag="inv", bufs=4)
                    nc.scalar.activation(lnc[:, 0:128], cs[:, 0:128], AF.Ln,
                                         scale=8.0)
                    inv = sb.tile([128, 256], F32, name=f"inva{l}_{kt}_{c}",
                                  tag="inv", bufs=4)
                    nc.scalar.activation(inv[:, 0:128], lnc[:, 0:128], AF.Exp,
                                         scale=-1.0)
                    nc.vector.tensor_mul(oT8[:, c * T + qw0:c * T + qw0 + 128],
                                         av[:, 0:128], inv[:, 0:128])

            # ---- O projection + residual -> s1, LN1 ----
            s1 = s_tile(f"s1_{l}")
            for i, (o, w) in enumerate(NTS):
                for mc in range(KC):
                    p = bank(f"po{l}_{mc}_{i}")
                    for kp in range(0, KC, 2):
                        _mm8(nc, p[:, 0:384],
                             wo83[:, kp:kp + 2, mc * 128:(mc + 1) * 128],
                             oT83[:, kp:kp + 2, o:o + w],
                             kp == 0, kp == KC - 2)
                    nc.vector.tensor_add(s1[:, mc * T + o:mc * T + o + w],
                                         p[:, 0:384],
                                         h[:, mc * T + o:mc * T + o + w])
            hmid = hpool(f"hmid{l}")
            hmid8 = h8pool(f"hmid8_{l}")
            layer_norm(f"ln1_{l}", s1, hmid, hmid8)
            hmid83 = hmid8.rearrange("p (c t) -> p c t", c=KC)

            # ---- FFN (half-outer; W1 tiles re-fetched per half) ----
            if 'ffn' in SKIP_SECT:
                nc.vector.memset(f18[:], 0.06)
            for i, (o, w) in enumerate(NTS):
                if 'ffn' in SKIP_SECT:
                    break
                for fc in range(FFC):
                    w1c = w1pool(f"w1_{l}_{i}_{fc}")
                    nc.sync.dma_start(w1c[:], dd['W18_d'][l, fc])
                    w1c3 = w1c.rearrange("p (c d) -> p c d", c=KC)
                    p = bank(f"pf1{l}_{fc}_{i}")
                    for kp in range(0, KC, 2):
                        _mm8(nc, p[:, 0:384],
                             w1c3[:, kp:kp + 2, :],
                             hmid83[:, kp:kp + 2, o:o + w],
                             kp == 0, kp == KC - 2)
                    nc.scalar.activation(f18[:, fc * T + o:fc * T + o + w],
                                         p[:, 0:384], AF.Gelu_apprx_tanh,
                                         bias=bf1c[:, fc:fc + 1], scale=QS)
            s2 = s_tile(f"s2_{l}")
            for i, (o, w) in enumerate(NTS):
                for mc in range(KC):
                    p = bank(f"pf2{l}_{mc}_{i}")
                    for fp in range(0, FFC, 2):
                        _mm8(nc, p[:, 0:384],
                             w2all3[:, fp:fp + 2, mc * 128:(mc + 1) * 128],
                             f183[:, fp:fp + 2, o:o + w],
                             fp == 0, fp == FFC - 2)
                    nc.vector.tensor_add(s2[:, mc * T + o:mc * T + o + w],
                                         p[:, 0:384],
                                         hmid[:, mc * T + o:mc * T + o + w])
            hout = hpool(f"hL{l}")
            hout8 = h8pool(f"hL8_{l}") if l < L - 1 else None
            layer_norm(f"ln2_{l}", s2, hout, hout8, corr_col=96 + 5 * l)
            pending_outb.append((hout, l))
            h = hout
            h8 = hout8

        while pending_outb:
            hs_prev, lprev = pending_outb.pop()
            emit_outb(hs_prev, lprev)
        nc.sync.dma_start(dd['out_d'][:], outb[:])


def _build_nc(n_layers=L, taps=(), with_bias=False, unit_norm=False, unit_gb=False):
    nc = bacc.Bacc("TRN2", target_bir_lowering=False, debug=False)

    dd = {}
    dd['xT_d'] = nc.dram_tensor("xT", [384, T], BF16, kind="ExternalInput")
    dd['fc1_d'] = nc.dram_tensor("fc1p", [384, D], BF16, kind="ExternalInput")
    dd['posT_d'] = nc.dram_tensor("posT", [D, T], BF16, kind="ExternalInput")
    dd['embg_d'] = nc.dram_tensor("emb_g", [D], F32, kind="ExternalInput")
    dd['embb_d'] = nc.dram_tensor("emb_b", [D], F32, kind="ExternalInput")
    dd['Wq_d'] = nc.dram_tensor("Wq", [L, D, D], BF16, kind="ExternalInput")
    dd['Wk_d'] = nc.dram_tensor("Wk", [L, D, D], BF16, kind="ExternalInput")
    dd['Wv_d'] = nc.dram_tensor("Wv", [L, D, D], BF16, kind="ExternalInput")
    dd['Wo_d'] = nc.dram_tensor("Wo", [L, D, D], BF16, kind="ExternalInput")
    dd['bq_d'] = nc.dram_tensor("bq", [L, D], F32, kind="ExternalInput")
    dd['bk_d'] = nc.dram_tensor("bk8", [L, D], F32, kind="ExternalInput")
    dd['bv_d'] = nc.dram_tensor("bv", [L, D], F32, kind="ExternalInput")
    dd['bo_d'] = nc.dram_tensor("bo", [L, D], F32, kind="ExternalInput")
    dd['l1g_d'] = nc.dram_tensor("ln1_g", [L, D], F32, kind="ExternalInput")
    dd['l1b_d'] = nc.dram_tensor("ln1_b", [L, D], F32, kind="ExternalInput")
    dd['l2g_d'] = nc.dram_tensor("ln2_g", [L, D], F32, kind="ExternalInput")
    dd['l2b_d'] = nc.dram_tensor("ln2_b", [L, D], F32, kind="ExternalInput")
    dd['W1r_d'] = nc.dram_tensor("W1r", [L, FFC, 128, D], BF16, kind="ExternalInput")
    dd['W2_d'] = nc.dram_tensor("W2", [L, FF, D], BF16, kind="ExternalInput")
    dd['bf1_d'] = nc.dram_tensor("bf1", [L, FF], F32, kind="ExternalInput")
    dd['bf2_d'] = nc.dram_tensor("bf2", [L, D], F32, kind="ExternalInput")
    dd['out_d'] = nc.dram_tensor("out", [128, 96], F32, kind="ExternalOutput")
    tap_d = {}
    for tname in taps:
        tap_d[tname] = nc.dram_tensor("tap_" + tname, [128, 6 * T], BF16,
                                      kind="ExternalOutput")

    with tile.TileContext(nc) as tc:
        _body(nc, tc, dd, n_layers, taps, tap_d, with_bias, unit_norm, unit_gb)
    nc.compile()
    return nc


def _body(nc, tc, dd, n_layers, taps, tap_d, with_bias, unit_norm, unit_gb):
    import contextlib
    ctx = contextlib.ExitStack()
    with ctx:
        sb = ctx.enter_context(tc.tile_pool(name="sb", bufs=1))
        ps = ctx.enter_context(tc.tile_pool(name="ps", bufs=1, space="PSUM"))

        # ---------------- constants ----------------
        ones_bf = sb.tile([128, 128], BF16, name="ones_bf", tag="const", bufs=4)
        nc.vector.memset(ones_bf[:], 1.0)
        eps12 = sb.tile([128, 1], F32, name="eps12", tag="const_e", bufs=2)
        nc.vector.memset(eps12[:], 1e-12)
        eps16 = sb.tile([128, 1], F32, name="eps16", tag="const_e2", bufs=2)
        nc.vector.memset(eps16[:], 1e-16)

        def col_tile(name, dram_row, n):
            t = sb.tile([128, n], F32, name=name, tag="cols", bufs=44)
            nc.sync.dma_start(t[:], dram_row.rearrange("(c p) -> p c", p=128))
            return t

        # 6 persistent big bf16 tiles [128, 6*768]; each layer uses 4 of them
        # (qT, kT, vtok, oT -> then f1 parts), rotated by 4 per layer so that
        # a layer's early writes never wait on the previous layer's F2 reads.
        bigs6 = [sb.tile([128, KC * T], BF16, name=f"bigp{i}", tag=f"pers{i}", bufs=1)
                 for i in range(6)]

        def layer_bigs(l):
            return [bigs6[(4 * l + i) % 6] for i in range(4)]

        def hpool(name):
            return sb.tile([128, KC * T], BF16, name=name, tag="hmid", bufs=2)

        def saved_tile(name):
            return sb.tile([128, KC * T], BF16, name=name, tag="saved", bufs=4)

        def s_tile(name):
            return sb.tile([128, KC * T], BF16, name=name, tag="spre", bufs=1)

        def wstr(name):
            return sb.tile([128, D], BF16, name=name, tag="wstr", bufs=10)

        def bank(name, w=384):
            return ps.tile([128, w], F32, name=name, tag="bank", bufs=8)

        def stat(name):
            return sb.tile([128, 384], F32, name=name, tag="stat", bufs=6)

        def statw(name):
            return sb.tile([128, D], F32, name=name, tag="statw", bufs=3)

        def stbf(name):
            return sb.tile([128, 384], BF16, name=name, tag="stbf", bufs=4)


        # ---------------- LayerNorm (token-half pipelined) ----------------
        def layer_norm(pref, s, g_col, gc, b_col, bc, out):
            """s: [128,4608] bf16 pre-LN sums; out: [128,4608] bf16.
            Processed independently per token half so consumers of half 0
            can start while half 1 is still in flight."""
            for ih, (o, w) in enumerate(NTS):
                S1 = bank(f"{pref}_S1_{ih}")
                S2 = bank(f"{pref}_S2_{ih}")
                for c in range(KC):
                    sq = stbf(f"{pref}_sq{ih}_{c}")
                    nc.gpsimd.tensor_mul(sq[:], s[:, c * T + o:c * T + o + w],
                                         s[:, c * T + o:c * T + o + w])
                    nc.tensor.matmul(S1[:], ones_bf[:], s[:, c * T + o:c * T + o + w],
                                     start=(c == 0), stop=(c == KC - 1))
                    nc.tensor.matmul(S2[:], ones_bf[:], sq[:],
                                     start=(c == 0), stop=(c == KC - 1))
                m2 = stat(f"{pref}_m2{ih}")
                nc.scalar.mul(m2[:], S2[:], 1.0 / D)
                var = stat(f"{pref}_var{ih}")
                mean = stat(f"{pref}_mean{ih}")
                nc.scalar.mul(mean[:], S1[:], 1.0 / D)
                nc.vector.tensor_mul(var[:], mean[:], mean[:])
                nc.vector.tensor_sub(var[:], m2[:], var[:])
                # rstd = exp(-0.5 * ln(var + eps))  (ACT, keeps DVE free)
                lnv = stat(f"{pref}_lnv{ih}")
                nc.scalar.activation(lnv[:], var[:], AF.Ln, bias=eps12[:])
                rstd = stat(f"{pref}_rstd{ih}")
                nc.scalar.activation(rstd[:], lnv[:], AF.Exp, scale=-0.5)
                for c in range(KC):
                    eng = nc.gpsimd if c >= 4 else nc.vector
                    t1 = stat(f"{pref}_t1_{ih}_{c}")
                    eng.tensor_sub(t1[:], s[:, c * T + o:c * T + o + w], mean[:])
                    if unit_gb:
                        # gamma==1/beta==0: the multiply writes the bf16 output
                        # directly, dropping the ACT affine from the chain
                        eng.tensor_mul(out[:, c * T + o:c * T + o + w], t1[:], rstd[:])
                    else:
                        eng.tensor_mul(t1[:], t1[:], rstd[:])
                        nc.scalar.activation(out[:, c * T + o:c * T + o + w], t1[:],
                                             AF.Identity, bias=b_col[:, bc:bc + 1],
                                             scale=g_col[:, gc:gc + 1])

        # ---------------- embed: fc1 + pos + LN ----------------
        xT_bf = [wstr(f"xT{c}") for c in range(3)]
        fc1_bf = [wstr(f"fc1w{c}") for c in range(3)]
        for c in range(3):
            nc.sync.dma_start(xT_bf[c][:], dd['xT_d'][c * 128:(c + 1) * 128, :])
            nc.sync.dma_start(fc1_bf[c][:], dd['fc1_d'][c * 128:(c + 1) * 128, :])
        posT = bigs6[0]
        for c in range(KC):
            nc.sync.dma_start(posT[:, c * T:(c + 1) * T],
                              dd['posT_d'][c * 128:(c + 1) * 128, :])
        embg = col_tile("embg", dd['embg_d'][:], KC)
        embb = col_tile("embb", dd['embb_d'][:], KC)

        s0 = s_tile("s_emb")
        for i, (o, w) in enumerate(NTS):
            for mc in range(KC):
                p = bank(f"emb_{mc}_{i}")
                for kc in range(3):
                    nc.tensor.matmul(p[:], fc1_bf[kc][:, mc * 128:(mc + 1) * 128],
                                     xT_bf[kc][:, o:o + w],
                                     start=(kc == 0), stop=(kc == 2))
                nc.vector.tensor_add(s0[:, mc * T + o:mc * T + o + w], p[:],
                                     posT[:, mc * T + o:mc * T + o + w])
        h = hpool("h0")
        layer_norm("ln_emb", s0, embg, 0, embb, 0, h)
        if 'h0' in taps:
            nc.sync.dma_start(tap_d['h0'][:], h[:])

        saved = []
        N2 = None if unit_norm else [bank(f"n2_{i}") for i in range(2)]
        outb = sb.tile([128, 96], F32, name="outb", tag="outb", bufs=1)
        nc.vector.memset(outb[:], 0.0)

        # ---------------- transformer layers ----------------
        for l in range(n_layers):
            bq = col_tile(f"bq{l}", dd['bq_d'][l], KC)
            bk = col_tile(f"bk{l}", dd['bk_d'][l], KC)
            l1g = col_tile(f"l1g{l}", dd['l1g_d'][l], KC)
            l1b = col_tile(f"l1b{l}", dd['l1b_d'][l], KC)
            l2g = col_tile(f"l2g{l}", dd['l2g_d'][l], KC)
            l2b = col_tile(f"l2b{l}", dd['l2b_d'][l], KC)
            bf1c = col_tile(f"bf1{l}", dd['bf1_d'][l], FFC)
            if with_bias:
                bvc = col_tile(f"bvc{l}", dd['bv_d'][l], KC)
                boc = col_tile(f"boc{l}", dd['bo_d'][l], KC)
                bf2c = col_tile(f"bf2c{l}", dd['bf2_d'][l], KC)

            # ---- Q^T, K^T (feature-major, K pre-scaled 1/8) ----
            lb = layer_bigs(l)
            qT = lb[0]
            kT = lb[1]
            for dst, W_d, bias_col, scl, wn in ((qT, dd['Wq_d'], bq, 1.0, 'q'),
                                                (kT, dd['Wk_d'], bk, 0.125, 'k')):
                w_bf = [wstr(f"w{wn}{l}_{c}") for c in range(KC)]
                for c in range(KC):
                    nc.sync.dma_start(w_bf[c][:], W_d[l, c * 128:(c + 1) * 128, :])
                for i, (o, w) in enumerate(NTS):
                    for mc in range(KC):
                        p = bank(f"p{wn}{l}_{mc}_{i}")
                        for kc in range(KC):
                            nc.tensor.matmul(p[:], w_bf[kc][:, mc * 128:(mc + 1) * 128],
                                             h[:, kc * T + o:kc * T + o + w],
                                             start=(kc == 0), stop=(kc == KC - 1))
                        if i == 0:
                            nc.scalar.activation(dst[:, mc * T + o:mc * T + o + w], p[:],
                                                 AF.Identity, bias=bias_col[:, mc:mc + 1],
                                                 scale=scl)
                        else:
                            nc.vector.tensor_scalar(
                                dst[:, mc * T + o:mc * T + o + w], p[:], scl,
                                bias_col[:, mc:mc + 1], mybir.AluOpType.mult,
                                mybir.AluOpType.add)

            # ---- V (token-major: [token_chunk rows, feature cols]) ----
            vtok = lb[2]
            wv_bf = [wstr(f"wv{l}_{c}") for c in range(KC)]
            for c in range(KC):
                nc.sync.dma_start(wv_bf[c][:], dd['Wv_d'][l, c * 128:(c + 1) * 128, :])
            for tch in range(KC):
                for i, (o, w) in enumerate(NTS):
                    p = bank(f"pv{l}_{tch}_{i}")
                    for kc in range(KC):
                        nc.tensor.matmul(p[:], h[:, kc * T + tch * 128:kc * T + (tch + 1) * 128],
                                         wv_bf[kc][:, o:o + w],
                                         start=(kc == 0), stop=(kc == KC - 1))
                    nc.vector.tensor_copy(vtok[:, tch * T + o:tch * T + o + w], p[:])

            # ---- attention (heads paired per feature chunk) ----
            # chunk-outer ordering: oT chunks complete in order c=0,1,2... so
            # the O-projection's kc-ascending accumulation overlaps attention.
            oT = lb[3]
            for c in range(KC):
                for si, (qo, kts) in enumerate(((0, (0, 1)), (256, (2, 3)))):
                    av = bank(f"av{l}_{si}_{c}", 256)
                    cs = bank(f"cs{l}_{si}_{c}", 256)
                    eTs = []
                    for ki, kt in enumerate(kts):
                        eT = sb.tile([128, 512], BF16, name=f"eT{l}_{si}_{c}_{ki}",
                                     tag="eT", bufs=5)
                        for j in range(2):
                            sp = bank(f"sp{l}_{si}_{c}_{ki}_{j}", 256)
                            nc.tensor.matmul(
                                sp[:],
                                kT[j * 64:j * 64 + 64, c * T + kt * 128:c * T + (kt + 1) * 128],
                                qT[j * 64:j * 64 + 64, c * T + qo:c * T + qo + 256],
                                start=True, stop=True)
                            nc.scalar.activation(eT[:, j * 256:(j + 1) * 256], sp[:], AF.Exp)
                        eTs.append(eT)
                    for j in range(2):
                        hh = 2 * c + j
                        r0 = j * 64
                        for ki in range(2):
                            nc.tensor.matmul(av[r0:r0 + 64, :],
                                             vtok[:, kts[ki] * T + hh * 64:kts[ki] * T + hh * 64 + 64],
                                             eTs[ki][:, j * 256:(j + 1) * 256],
                                             start=(ki == 0), stop=(ki == 1),
                                             tile_position=(0, r0))
                        for ki in range(2):
                            nc.tensor.matmul(cs[r0:r0 + 64, :], ones_bf[:, 0:64],
                                             eTs[ki][:, j * 256:(j + 1) * 256],
                                             start=(ki == 0), stop=(ki == 1),
                                             tile_position=(0, r0))
                    # inv = exp(-ln(cs)) on ACT (frees DVE); multiply on DVE (PSUM)
                    lnc = sb.tile([128, 256], F32, name=f"lnc{l}_{si}_{c}",
                                  tag="inv", bufs=4)
                    nc.scalar.activation(lnc[:], cs[:], AF.Ln)
                    inv = sb.tile([128, 256], F32, name=f"inv{l}_{si}_{c}",
                                  tag="inv", bufs=4)
                    nc.scalar.activation(inv[:], lnc[:], AF.Exp, scale=-1.0)
                    nc.vector.tensor_mul(oT[:, c * T + qo:c * T + qo + 256], av[:], inv[:])
                for kt in (4, 5):
                    qw0 = 512 + (kt - 4) * 128
                    col0 = (kt - 4) * 128
                    eT = sb.tile([128, 512], BF16, name=f"eTa{l}_{kt}_{c}",
                                 tag="eT", bufs=5)
                    for j in range(2):
                        sp = bank(f"spa{l}_{kt}_{c}_{j}", 256)
                        nc.tensor.matmul(
                            sp[:],
                            kT[j * 64:j * 64 + 64, c * T + kt * 128:c * T + (kt + 1) * 128],
                            qT[j * 64:j * 64 + 64, c * T + 512:c * T + 768],
                            start=True, stop=True)
                        nc.scalar.activation(eT[:, j * 256:(j + 1) * 256], sp[:], AF.Exp)
                    av = bank(f"ava{l}_{kt}_{c}", 128)
                    cs = bank(f"csa{l}_{kt}_{c}", 128)
                    for j in range(2):
                        hh = 2 * c + j
                        r0 = j * 64
                        nc.tensor.matmul(av[r0:r0 + 64, :],
                                         vtok[:, kt * T + hh * 64:kt * T + hh * 64 + 64],
                                         eT[:, j * 256 + col0:j * 256 + col0 + 128],
                                         start=True, stop=True, tile_position=(0, r0))
                        nc.tensor.matmul(cs[r0:r0 + 64, :], ones_bf[:, 0:64],
                                         eT[:, j * 256 + col0:j * 256 + col0 + 128],
                                         start=True, stop=True, tile_position=(0, r0))
                    lnc = sb.tile([128, 256], F32, name=f"lnca{l}_{kt}_{c}",
                                  tag="inv", bufs=4)
                    nc.scalar.activation(lnc[:, 0:128], cs[:], AF.Ln)
                    inv = sb.tile([128, 256], F32, name=f"inva{l}_{kt}_{c}",
                                  tag="inv", bufs=4)
                    nc.scalar.activation(inv[:, 0:128], lnc[:, 0:128], AF.Exp, scale=-1.0)
                    nc.vector.tensor_mul(oT[:, c * T + qw0:c * T + qw0 + 128],
                                         av[:], inv[:, 0:128])

            if with_bias:
                for c in range(KC):
                    nc.scalar.activation(oT[:, c * T:(c + 1) * T],
                                         oT[:, c * T:(c + 1) * T], AF.Identity,
                                         bias=bvc[:, c:c + 1])

            # ---- O projection + residual -> s1, LN1 -> hmid ----
            wo_bf = [wstr(f"wo{l}_{c}") for c in range(KC)]
            for c in range(KC):
                nc.sync.dma_start(wo_bf[c][:], dd['Wo_d'][l, c * 128:(c + 1) * 128, :])
            s1 = s_tile(f"s1_{l}")
            for i, (o, w) in enumerate(NTS):
                for mc in range(KC):
                    p = bank(f"po{l}_{mc}_{i}")
                    for kc in range(KC):
                        nc.tensor.matmul(p[:], wo_bf[kc][:, mc * 128:(mc + 1) * 128],
                                         oT[:, kc * T + o:kc * T + o + w],
                                         start=(kc == 0), stop=(kc == KC - 1))
                    nc.vector.tensor_add(s1[:, mc * T + o:mc * T + o + w], p[:],
                                         h[:, mc * T + o:mc * T + o + w])
                    if with_bias:
                        nc.scalar.activation(s1[:, mc * T + o:mc * T + o + w],
                                             s1[:, mc * T + o:mc * T + o + w],
                                             AF.Identity, bias=boc[:, mc:mc + 1])
            hmid = hpool(f"hmid{l}")
            layer_norm(f"ln1_{l}", s1, l1g, 0, l1b, 0, hmid)

            # ---- FFN ----
            w2all = sb.tile([128, FFC * T], BF16, name=f"w2all{l}", tag="w2all", bufs=1)
            for fc in range(FFC):
                nc.sync.dma_start(w2all[:, fc * T:(fc + 1) * T],
                                  dd['W2_d'][l, fc * 128:(fc + 1) * 128, :])
            f1 = lb
            for fc in range(FFC):
                w1c = wstr(f"w1_{l}_{fc}")
                nc.sync.dma_start(w1c[:], dd['W1r_d'][l, fc])
                ftile, foff = f1[fc // 6], (fc % 6) * T
                for i, (o, w) in enumerate(NTS):
                    p = bank(f"pf1{l}_{fc}_{i}")
                    for kc in range(KC):
                        nc.tensor.matmul(p[:], w1c[:, kc * 128:(kc + 1) * 128],
                                         hmid[:, kc * T + o:kc * T + o + w],
                                         start=(kc == 0), stop=(kc == KC - 1))
                    nc.scalar.activation(ftile[:, foff + o:foff + o + w], p[:],
                                         AF.Gelu_apprx_tanh, bias=bf1c[:, fc:fc + 1])
            s2 = s_tile(f"s2_{l}")
            for i, (o, w) in enumerate(NTS):
                for mc in range(KC):
                    p = bank(f"pf2{l}_{mc}_{i}")
                    for fc in range(FFC):
                        ftile, foff = f1[fc // 6], (fc % 6) * T
                        nc.tensor.matmul(
                            p[:], w2all[:, fc * T + mc * 128:fc * T + (mc + 1) * 128],
                            ftile[:, foff + o:foff + o + w],
                            start=(fc == 0), stop=(fc == FFC - 1))
                    nc.vector.tensor_add(s2[:, mc * T + o:mc * T + o + w], p[:],
                                         hmid[:, mc * T + o:mc * T + o + w])
                    if with_bias:
                        nc.scalar.activation(s2[:, mc * T + o:mc * T + o + w],
                                             s2[:, mc * T + o:mc * T + o + w],
                                             AF.Identity, bias=bf2c[:, mc:mc + 1])
            hout = saved_tile(f"hL{l}")
            layer_norm(f"ln2_{l}", s2, l2g, 0, l2b, 0, hout)
            if unit_norm:
                for c in range(KC):
                    base = (l * KC + c) * 4
                    nc.vector.reduce_sum(
                        outb[:, base:base + 2],
                        hout[:, c * T:c * T + 512].rearrange("p (s q) -> p s q", s=2),
                        axis=AX.X)
                    nc.vector.reduce_sum(
                        outb[:, base + 2:base + 4],
                        hout[:, c * T + 512:c * T + 768].rearrange("p (s q) -> p s q", s=2),
                        axis=AX.X)
            if not unit_norm:
                # accumulate squared-norm contribution for the final cosine stage
                for c in range(KC):
                    for i, (o, w) in enumerate(NTS):
                        sq = stbf(f"fin_sq{l}_{c}_{i}")
                        nc.gpsimd.tensor_mul(sq[:], hout[:, c * T + o:c * T + o + w],
                                             hout[:, c * T + o:c * T + o + w])
                        nc.tensor.matmul(N2[i][:], ones_bf[:], sq[:],
                                         start=(l == 0 and c == 0),
                                         stop=(l == n_layers - 1 and c == KC - 1))
            saved.append(hout)
            h = hout
            if f"h{l + 1}" in taps:
                nc.sync.dma_start(tap_d[f"h{l + 1}"][:], hout[:])

        # ---------------- final: weighted per-seq sums ----------------
        if unit_norm:
            pass  # raw sums emitted inside the layer loop
        else:
            invn = statw("invn")
            for i, (o, w) in enumerate(NTS):
                sd = stat(f"fin_sd{i}")
                nc.scalar.activation(sd[:], N2[i][:], AF.Sqrt, bias=eps16[:])
                nc.vector.reciprocal(invn[:, o:o + w], sd[:])
            for li, hs in enumerate(saved):
                for c in range(KC):
                    wt = statw(f"fin_w{li}_{c}")
                    eng = nc.vector if (c % 2 == 0) else nc.gpsimd
                    eng.tensor_mul(wt[:], hs[:, c * T:(c + 1) * T], invn[:])
                    base = (li * KC + c) * 4
                    nc.vector.reduce_sum(outb[:, base:base + 2],
                                         wt[:, 0:512].rearrange("p (s q) -> p s q", s=2),
                                         axis=AX.X)
                    nc.vector.reduce_sum(outb[:, base + 2:base + 4],
                                         wt[:, 512:768].rearrange("p (s q) -> p s q", s=2),
                                         axis=AX.X)
        nc.sync.dma_start(dd['out_d'][:], outb[:])


def _prep_in_maps_fast(inputs):
    f = lambda k: np.ascontiguousarray(np.asarray(inputs[k], np.float32))
    bf = lambda a: np.ascontiguousarray(np.asarray(a, ml_dtypes.bfloat16))

    def q8(a, scale):
        a = np.asarray(a, np.float32) * scale
        a = np.clip(a, -240.0, 240.0)
        return np.ascontiguousarray(a.astype(ml_dtypes.float8_e4m3))

    ctx_e, asp_e = f('context_inputs_embeds'), f('aspect_inputs_embeds')
    fc1_w, fc1_b = f('fc1_w'), f('fc1_b')
    pos = f('pos_emb')

    fc1p = np.zeros((384, D), np.float32)
    fc1p[:EMB] = fc1_w
    posT = np.concatenate([pos[:256].T, pos[:256].T, pos[:128].T, pos[:128].T],
                          axis=1) + fc1_b[:, None]

    def wqkvo(key, scale):
        W = f(key)  # [L, D, D]
        W8 = W.reshape(L, KC, 128, D).transpose(0, 2, 1, 3).reshape(L, 128, KC * D)
        return q8(W8, scale)

    W1 = f('W1')
    W1r = W1.reshape(L, KC, 128, FFC, 128).transpose(0, 3, 2, 1, 4).reshape(L, FFC, 128, D)
    W2 = f('W2')
    W28 = W2.reshape(L, FFC, 128, D).transpose(0, 2, 1, 3).reshape(L, 128, FFC * D)

    common = {
        'fc1p': bf(fc1p), 'posT': bf(posT),
        'bq': f('bq'), 'bk8': f('bk') * 0.125, 'bf1': f('bf1'),
        'Wq8': wqkvo('Wq', 64.0), 'Wk8': wqkvo('Wk', 64.0),
        'Wv8': wqkvo('Wv', 64.0), 'Wo8': wqkvo('Wo', 8.0),
        'W18': q8(W1r, 64.0), 'W28': q8(W28, 1.0),
    }
    in_maps = []
    for i in range(8):
        xT = np.zeros((384, T), np.float32)
        xT[:EMB, 0:256] = ctx_e[2 * i].T
        xT[:EMB, 256:512] = ctx_e[2 * i + 1].T
        xT[:EMB, 512:640] = asp_e[2 * i].T
        xT[:EMB, 640:768] = asp_e[2 * i + 1].T
        in_maps.append({**common, 'xT': bf(xT)})
    return in_maps


def _prep_in_maps(inputs):
    f = lambda k: np.ascontiguousarray(np.asarray(inputs[k], np.float32))
    bf = lambda a: np.ascontiguousarray(np.asarray(a, ml_dtypes.bfloat16))
    ctx_e, asp_e = f('context_inputs_embeds'), f('aspect_inputs_embeds')
    fc1_w, fc1_b = f('fc1_w'), f('fc1_b')
    pos = f('pos_emb')

    fc1p = np.zeros((384, D), np.float32)
    fc1p[:EMB] = fc1_w
    posT = np.concatenate([pos[:256].T, pos[:256].T, pos[:128].T, pos[:128].T],
                          axis=1) + fc1_b[:, None]
    W1 = f('W1')
    # [L, FFC, 128, 768]: W1r[l, fc, p, kc*128+c2] = W1[l, kc*128+p, fc*128+c2]
    W1r = W1.reshape(L, KC, 128, FFC, 128).transpose(0, 3, 2, 1, 4).reshape(L, FFC, 128, D)
    common = {
        'fc1p': bf(fc1p), 'posT': bf(posT),
        'emb_g': f('emb_g'), 'emb_b': f('emb_b'),
        'Wq': bf(f('Wq')), 'Wk': bf(f('Wk')), 'Wv': bf(f('Wv')), 'Wo': bf(f('Wo')),
        'bq': f('bq'), 'bk8': f('bk') * 0.125, 'bv': f('bv'), 'bo': f('bo'),
        'ln1_g': f('ln1_g'), 'ln1_b': f('ln1_b'),
        'ln2_g': f('ln2_g'), 'ln2_b': f('ln2_b'),
        'W1r': bf(W1r), 'W2': bf(f('W2')), 'bf1': f('bf1'), 'bf2': f('bf2'),
    }
    in_maps = []
    for i in range(8):
        xT = np.zeros((384, T), np.float32)
        xT[:EMB, 0:256] = ctx_e[2 * i].T
        xT[:EMB, 256:512] = ctx_e[2 * i + 1].T
        xT[:EMB, 512:640] = asp_e[2 * i].T
        xT[:EMB, 640:768] = asp_e[2 * i + 1].T
        in_maps.append({**common, 'xT': bf(xT)})
    return in_maps


def _postprocess(results, unit_norm=False):
    scale = 1.0 / 3072.0 if unit_norm else 1.0
    out = np.zeros(16, np.float32)
    for i, r in enumerate(results):
        A = r['out'].reshape(128, 24, 4)
        for j in range(2):
            out[2 * i + j] = scale * float(np.sum(
                A[:, :, 2 + j].astype(np.float64) * A[:, :, j].astype(np.float64)))
    return out


def get_nc(n_layers=L, taps=(), with_bias=False, unit_norm=False, unit_gb=False,
           fast=False):
    key = (n_layers, tuple(taps), with_bias, unit_norm, unit_gb, fast)
    if key not in _NC_CACHE:
        if fast:
            _NC_CACHE[key] = _build_nc_fast()
        else:
            _NC_CACHE[key] = _build_nc(n_layers, taps, with_bias, unit_norm, unit_gb)
    return _NC_CACHE[key]


def _build_flags(inputs):
    wb = any(float(np.abs(np.asarray(inputs[k])).max()) > 0
             for k in ('bv', 'bo', 'bf2'))
    ugb = all(np.all(np.asarray(inputs[g]) == 1.0) and
              np.all(np.asarray(inputs[b]) == 0.0)
              for g, b in (('emb_g', 'emb_b'), ('ln1_g', 'ln1_b'),
                           ('ln2_g', 'ln2_b')))
    fast = ugb and not wb
    if fast:
        return dict(fast=True)
    return dict(with_bias=wb, unit_norm=False, unit_gb=ugb)


def _postprocess_fast(results):
    # outb[:, :96] holds per-seq sums of hr = s*rstd (offset-carrying);
    # outb[0, 96:116] holds per-layer segment sums of mean*rstd, which the
    # host subtracts to recover sums of the true unit-norm LN outputs.
    # Cosine normalizer is the constant 1/3072.
    out = np.zeros(16, np.float32)
    for i, r in enumerate(results):
        raw = r['out']
        A = raw[:, :96].reshape(128, 4, 6, 4).astype(np.float64)
        c5 = raw[0, 96:116].reshape(4, 5).astype(np.float64)
        corr = np.zeros((4, 4))
        corr[:, 0] = c5[:, 0]            # ctx0 = h0[0:256]
        corr[:, 1] = c5[:, 1] + c5[:, 2]  # ctx1 = h0[256:384] + h1[0:128]
        corr[:, 2] = c5[:, 3]            # asp0 = h1[128:256]
        corr[:, 3] = c5[:, 4]            # asp1 = h1[256:384]
        A = A - corr[None, :, None, :]
        for j in range(2):
            out[2 * i + j] = (1.0 / 3072.0) * float(
                np.sum(A[:, :, :, 2 + j] * A[:, :, :, j]))
    return out


def kernel(**inputs):
    flags = _build_flags(inputs)
    fast = flags.get('fast', False)
    nc = get_nc(**flags)
    in_maps = _prep_in_maps_fast(inputs) if fast else _prep_in_maps(inputs)
    last_err = None
    for attempt in range(3):
        try:
            res = run_bass_kernel_spmd(nc, in_maps, CORES)
            if fast:
                return _postprocess_fast(res.results)
            return _postprocess(res.results, False)
        except Exception as e:  # transient NRT_EXEC_UNIT_UNRECOVERABLE flakes
            last_err = e
            import time
            time.sleep(15)
    raise last_err


if __name__ == "__main__":
    d = np.load('/root/problem/inputs_cache.npz')
    out = kernel(**{k: d[k] for k in d.files})
    ref = np.load('/root/problem/ref_out.npy')
    rel = np.abs(out - ref) / np.abs(ref)
    print("out:", out)
    print("rel err:", rel.max())

